# revision 75
# baseline (speedup 1.0000x reference)
"""MultiHeadAttention TRN2 kernel: B=2, S=2048, D=1024, H=16, DK=64, 8 cores.

Sharding: core c handles batch b=c//4 and heads hg=(c%4)*4 .. +3 (data + head
parallel). Projections are column-split by head; out-proj row-split; the
all-reduce after out-proj is done on host (sum of 4 partials per batch).

All activations/weights stream HBM<->SBUF as bf16 (host converts), halving
DMA on the serial DMA-engine resource. Matmul inputs are bf16 (1 cycle/row at
any moving size) except qT/kT which stay f32r for exp-input precision.

Device dataflow (per core):
  qT/kT = (w-slice).T @ QT/KT      -> [feat 128 (2 heads), seq] f32r, 256-wide
  v     = VT.T @ wv-slice          -> natural [kpos, 4*64] chunks -> v_all bf16
                                      [kpos, head*16*65] with ones col (den)
  scoresT[kpos,q] = kT-chunk.T @ qT  (K=64, both heads packed in one
                                      [128,1024] 2-bank PSUM tile)
  expT = exp(scoresT/8)            -> bf16 SBUF (ACT, the throughput floor)
  ctx[q, 65] += expT-slice.T @ v-chunk   (F=65 bf16, PSUM accum over kpos;
                                          col 64 accumulates the denominator)
  ctx_sb = ctx * recip(den)        -> [q, 128(2 heads)] f32 per qsub
  ctxT = PE-transpose(ctx_sb)      -> ctxT_sb [feat, q] bf16
  out[q, Dout] = ctxT.T @ wo       -> 256-wide chunks -> bf16 partial out

The whole thing is software-pipelined at DMA-chunk granularity: the hp0
weight halves + K s0 + Q s0 form a minimal first-exp prefix on the sync
queue (first exp at ~12us), then the remaining K/V/Q chunks stream in while
one global stream of 128 scores->exp units runs; ctx matmuls trail the exp
stream by 13..18 units during sweep 0 (V still arriving) then catch up to a
lag of 2; projections / out-proj / norms / transposes are injected between
units so the in-order PE never waits on data that hasn't arrived. The exp
stream runs back-to-back (1038 ns/tile) through the ACT-bound middle.

PSUM (8 banks): scA/scB [128,1024] x2 banks each (scores/exp dbuf) | ctxA,
ctxB [128,455] (7 of 8 per-sweep [128,65] accumulators, parity-alternating) |
ctxC [128,512] (8th accumulator per parity) | projC [128,512] (2 rotating
256-wide slots for q/k/v-proj, out-proj and ctx transposes).

Bias handling (exact): bq added on device (per-partition add in qT layout);
bk dropped (softmax shift-invariance); bv and bo folded on host as
out += bv @ wo.T + bo (softmax weights sum to 1).
"""

from contextlib import ExitStack

import numpy as np

B, S, D, H, DK = 2, 2048, 1024, 16, 64
NCORES = 8
HPC = H // (NCORES // B)      # heads per core = 4
R = HPC * DK                  # local feats = 256
NKC = S // 128                # 128-wide k chunks = 16
VW = 65                       # v chunk width (64 + ones col)

_CACHE = {}
_LAST_IN_MAPS = None


def _build():
    import concourse.mybir as mybir
    import concourse.tile as tile
    from concourse import bacc

    f32 = mybir.dt.float32
    f32r = mybir.dt.float32r
    bf16 = mybir.dt.bfloat16
    Exp = mybir.ActivationFunctionType.Exp
    Add = mybir.AluOpType.add
    Mult = mybir.AluOpType.mult

    nc = bacc.Bacc(
        "TRN2", target_bir_lowering=False, debug=False,
        enable_asserts=True, num_devices=NCORES,
    )

    QT_d = nc.dram_tensor("QT", [D, S], bf16, kind="ExternalInput").ap()
    KT_d = nc.dram_tensor("KT", [D, S], bf16, kind="ExternalInput").ap()
    VT_d = nc.dram_tensor("VT", [D, S], bf16, kind="ExternalInput").ap()
    # weights host-permuted to exact SBUF layout: wq/wk as [128, hp, d, 128]
    # (per-hp halves load separately, 2KB-contiguous rows), wv as [128, d, 256]
    wqP_d = nc.dram_tensor("wqP", [128, 2 * 8 * 128], bf16,
                           kind="ExternalInput").ap()
    wkP_d = nc.dram_tensor("wkP", [128, 2 * 8 * 128], bf16,
                           kind="ExternalInput").ap()
    wvP_d = nc.dram_tensor("wvP", [128, 8 * 256], bf16,
                           kind="ExternalInput").ap()
    woT_d = nc.dram_tensor("woT", [R, D], bf16, kind="ExternalInput").ap()
    bq_d = nc.dram_tensor("bq", [R, 1], f32, kind="ExternalInput").ap()
    id_d = nc.dram_tensor("ident", [128, 128], f32, kind="ExternalInput").ap()
    out_d = nc.dram_tensor("OUT", [S, D], bf16, kind="ExternalOutput").ap()

    with tile.TileContext(nc) as tc, ExitStack() as ctx:
        sb = ctx.enter_context(tc.tile_pool(name="sb", bufs=1))
        qin = ctx.enter_context(tc.tile_pool(name="qin", bufs=6))
        kin = ctx.enter_context(tc.tile_pool(name="kin", bufs=8))
        vin = ctx.enter_context(tc.tile_pool(name="vin", bufs=6))
        expp = ctx.enter_context(tc.tile_pool(name="expp", bufs=20))
        cxp = ctx.enter_context(tc.tile_pool(name="cxp", bufs=2))
        osb = ctx.enter_context(tc.tile_pool(name="osb", bufs=4))
        psum = ctx.enter_context(tc.tile_pool(name="psum", bufs=1, space="PSUM"))

        # ---- persistent PSUM containers (8 banks exactly) ----
        # PSUM accumulation groups are zero-region (= bank) granular: a
        # start_tensor_calc matmul zeroes its whole bank, so each bank holds
        # exactly one live group. Scores halves are full banks; the 4 ctx
        # accumulators of one hh live in one bank as a single group; proj /
        # out-proj / transpose rotate through two whole-bank slots.
        sc_ps = [psum.tile([128, 1024], f32, name=f"sc{i}") for i in range(2)]
        ctxH = [psum.tile([128, 260], f32, name=f"ctxh{i}") for i in range(2)]
        projAB = [psum.tile([128, 512], f32, name=f"proj{i}") for i in range(2)]

        cnt = {"p": 0}

        def p_slot(w):
            # rotating whole-bank psum slot for q/k/v-proj, out-proj and
            # transposes; overlapping-view hazards serialize reuse
            i = cnt["p"]; cnt["p"] += 1
            return projAB[i % 2][:, 0:w]

        # ---- persistent SBUF ----
        wq_sb = sb.tile([128, 8 * R], bf16)   # (hp, d) block at 1024*hp+128*d
        wk_sb = sb.tile([128, 8 * R], bf16)
        wv_sb = sb.tile([128, 8 * R], bf16)   # D-chunk d at cols [R*d : +R]
        wo_sb = [sb.tile([128, D], bf16, name=f"wo_sb{cn}") for cn in range(2)]
        bq_sb = sb.tile([128, 2], f32)
        id_sb = sb.tile([128, 128], f32)

        qT_sb = [sb.tile([128, S], bf16, name=f"qT_sb{hp}") for hp in range(2)]
        kT_sb = [sb.tile([128, S], bf16, name=f"kT_sb{hp}") for hp in range(2)]
        v_all = sb.tile([128, HPC * NKC * VW], bf16)  # (h, c) at (h*NKC+c)*VW
        ctxT_sb = [sb.tile([128, S], bf16, name=f"ctxT_sb{cn}") for cn in range(2)]

        onecol = sb.tile([128, 1], f32)
        nc.vector.memset(onecol[:], 1.0)
        vv = v_all.rearrange("p (n c) -> p n c", c=VW)[:, :, 64:65].rearrange(
            "p n c -> p (n c)")
        nc.vector.tensor_copy(vv, onecol[:].broadcast_to((128, HPC * NKC)))

        def w_load(w_sb, w_d, hp=None):
            if hp is None:
                nc.sync.dma_start(w_sb[:], w_d[:, :])
            else:
                nc.sync.dma_start(w_sb[:, 1024 * hp:1024 * (hp + 1)],
                                  w_d[:, 1024 * hp:1024 * (hp + 1)])

        # staging tiles: one [128, 2048] bf16 tile covers 4 d-chunks x 512
        # seq; a (tensor, sblk) pair = 2 tiles (d 0-3, d 4-7)
        stage = {}

        def chunk_load(src, pool, tag, sblk):
            tiles = []
            for hf in range(2):
                t = pool.tile([128, 2048], bf16, name=tag, tag=tag)
                nc.sync.dma_start(
                    t.rearrange("p (d s) -> p d s", d=4),
                    src.rearrange("(d p) s -> p d s", p=128)[
                        :, 4 * hf:4 * hf + 4, 512 * sblk:512 * (sblk + 1)])
                tiles.append(t)
            stage[(tag, sblk)] = ("std", tiles)

        def first_load(src, pool, tag):
            # s0 of K/Q as two all-d x 256-col chunks so the first proj
            # tile only waits for half the data
            tiles = []
            for half in range(2):
                t = pool.tile([128, 2048], bf16, name=tag, tag=tag)
                nc.sync.dma_start(
                    t.rearrange("p (d s) -> p d s", d=8),
                    src.rearrange("(d p) s -> p d s", p=128)[
                        :, :, 256 * half:256 * (half + 1)])
                tiles.append(t)
            stage[(tag, 0)] = ("first", tiles)

        def staged(tag, sblk, d, cols):
            # d-chunk d of sblk, column slice `cols` within the 512-wide sblk
            ent = stage[(tag, sblk)]
            if ent[0] == "first":
                half = cols[0] // 256
                off = cols[0] - 256 * half
                return ent[1][half][
                    :, 256 * d + off:256 * d + off + cols[1] - cols[0]]
            t = ent[1][d // 4]
            base = 512 * (d % 4)
            return t[:, base + cols[0]:base + cols[1]]

        # ---- projection tiles ----
        def qk_proj(tag, w_sb, dst_sb, hp, j, bias):
            # (hp, j): 256 seq cols [256j : 256j+256] of head-pair hp
            sblk, half = j // 2, j % 2
            cols = (256 * half, 256 * half + 256)
            p_ps = p_slot(256)
            for d in range(8):
                nc.tensor.matmul(
                    p_ps[:],
                    w_sb[:, 1024 * hp + 128 * d:1024 * hp + 128 * (d + 1)],
                    staged(tag, sblk, d, cols), start=(d == 0), stop=(d == 7))
            dst = dst_sb[hp][:, 256 * j:256 * (j + 1)]
            if bias:
                nc.vector.tensor_scalar(
                    dst, p_ps[:], bq_sb[:, hp:hp + 1], None, op0=Add)
            else:
                nc.vector.tensor_copy(dst, p_ps[:])

        def v_proj(c):
            # kpos chunk c (128 rows): out [kpos, 256 feats] -> v_all slices
            sblk, sub = c // 4, c % 4
            cols = (128 * sub, 128 * sub + 128)
            v_ps = p_slot(256)
            for d in range(8):
                nc.tensor.matmul(
                    v_ps[:], staged("v", sblk, d, cols),
                    wv_sb[:, R * d:R * (d + 1)], start=(d == 0), stop=(d == 7))
            va = v_all.rearrange("p (h n c) -> p h n c", h=HPC, n=NKC)
            nc.vector.tensor_copy(
                va[:, :, c:c + 1, 0:64],
                v_ps[:].rearrange("p (h n c) -> p h n c", h=HPC, n=1))

        # ---- attention sweep pieces (scores/exp stream + trailing ctx) ----
        exp_ring = {}

        def scores_exp(qvb, hp, c):
            s_ps = sc_ps[c % 2]
            for hh in range(2):
                nc.tensor.matmul(
                    s_ps[:, 512 * hh:512 * (hh + 1)],
                    kT_sb[hp][64 * hh:64 * (hh + 1), 128 * c:128 * (c + 1)],
                    qT_sb[hp][64 * hh:64 * (hh + 1), 512 * qvb:512 * (qvb + 1)],
                    start=True, stop=True)
            expT = expp.tile([128, 1024], bf16, name="expT")
            nc.scalar.activation(expT[:], s_ps[:], Exp, scale=0.125)
            exp_ring[(qvb, hp, c)] = expT

        def ctx_mm(qvb, hp, c):
            expT = exp_ring.pop((qvb, hp, c))
            for hh in range(2):
                gh = HPC // 2 * hp + hh
                for qs in range(4):
                    nc.tensor.matmul(
                        ctxH[hh][0:128, 65 * qs:65 * qs + VW],
                        expT[:, 512 * hh + 128 * qs:512 * hh + 128 * (qs + 1)],
                        v_all[:, (gh * NKC + c) * VW:(gh * NKC + c + 1) * VW],
                        start=(c == 0 and qs == 0),
                        stop=(c == NKC - 1 and qs == 3))

        ctx_stage = {}

        def norm_hh(qvb, hp, hh):
            # drain one ctxH bank: a strided recip over the 4 denominator
            # columns + one strided multiply into the (qs, hh, 64) staging
            # tile cs
            if hh == 0:
                ctx_stage[(qvb, hp)] = cxp.tile(
                    [128, 512], f32, name="ctxs", tag=f"ctxs{hp}")
            cs = ctx_stage[(qvb, hp)]
            t3 = ctxH[hh].rearrange("p (qs w) -> p qs w", w=VW)
            rb = cxp.tile([128, 4], f32, name="rb", tag=f"rb{hh}")
            nc.vector.reciprocal_approx_fast(
                out=rb[:], in_=t3[:, :, 64:65].rearrange("p a b -> p (a b)"))
            nc.vector.tensor_mul(
                cs.rearrange("p (qs hh f) -> p qs hh f", qs=4, hh=2)[
                    :, :, hh, :],
                t3[:, :, 0:64],
                rb.rearrange("p (a b) -> p a b", b=1).broadcast_to(
                    (128, 4, 64)))

        def norm(qvb, hp):
            norm_hh(qvb, hp, 0)
            norm_hh(qvb, hp, 1)

        def transpose(qvb, hp, qs, tail=False):
            cs = ctx_stage[(qvb, hp)]
            lhsT = cs[:, 128 * qs:128 * (qs + 1)]
            tp = t_slot(128) if tail else p_slot(128)
            nc.tensor.matmul(tp, lhsT, id_sb[:], is_transpose=True)
            # after the last exp ACT is idle; split evacs across ACT and DVE
            dst = ctxT_sb[hp][:, 512 * qvb + 128 * qs:512 * qvb + 128 * (qs + 1)]
            if tail and qs % 2 == 0:
                nc.scalar.copy(dst, tp)
            else:
                nc.vector.tensor_copy(dst, tp)
            if qs == 3:
                del ctx_stage[(qvb, hp)]

        # after the final exp the score banks are free: the tail out-proj /
        # transposes rotate over 4 whole-bank psum slots instead of 2
        tail_slots = [lambda w: projAB[0][:, 0:w], lambda w: projAB[1][:, 0:w],
                      lambda w: sc_ps[0][:, 0:w], lambda w: sc_ps[0][:, 512:512 + w],
                      lambda w: sc_ps[1][:, 0:w], lambda w: sc_ps[1][:, 512:512 + w]]

        def t_slot(w):
            i = cnt["p"]; cnt["p"] += 1
            return tail_slots[i % 6](w)

        def out_proj(qvb, qs, tail=False):
            o_sb = osb.tile([128, D], bf16, name="o_sb")
            qcols = (512 * qvb + 128 * qs, 512 * qvb + 128 * (qs + 1))
            for dc in range(2):
                o_ps = t_slot(512) if tail else p_slot(512)
                for cn in range(2):
                    nc.tensor.matmul(
                        o_ps[:], ctxT_sb[cn][:, qcols[0]:qcols[1]],
                        wo_sb[cn][:, 512 * dc:512 * (dc + 1)],
                        start=(cn == 0), stop=(cn == 1))
                if tail and dc % 2 == 0:
                    nc.scalar.copy(o_sb[:, 512 * dc:512 * (dc + 1)], o_ps[:])
                else:
                    nc.vector.tensor_copy(o_sb[:, 512 * dc:512 * (dc + 1)],
                                          o_ps[:])
                # half-store right after its evac, alternating queues so the
                # final stores drain two DGE pipelines in parallel; the last
                # store goes on sync (no gpsimd Q7 launch on the tail)
                eng = nc.gpsimd if dc == 0 else nc.sync
                eng.dma_start(
                    out_d[qcols[0]:qcols[1], 512 * dc:512 * (dc + 1)],
                    o_sb[:, 512 * dc:512 * (dc + 1)])

        # ================= emission schedule =================
        # One global stream of 128 exp units ((qvb, hp) sweeps, kchunk
        # minor). ctx matmuls trail by 13 units during sweep 0 (V still
        # streaming), then catch up 2-per-unit to a lag of 2 so the tail
        # stays short.
        UNITS = [(s // 2, s % 2, c) for s in range(8) for c in range(NKC)]
        NU = len(UNITS)
        pre = {u: [] for u in range(NU + 16)}

        def at(u, fn, *a):
            pre[u].append((fn, a))

        def wo_load():
            for cn in range(2):
                nc.sync.dma_start(wo_sb[cn][:], woT_d[128 * cn:128 * (cn + 1), :])

        # PE warmup: keep the tensor engine busy (and its p-state ramp hot)
        # through the DMA-bound prologue; calibrated to end near Ks0 arrival
        wsc = sb.tile([128, 512], bf16)
        nc.vector.memset(wsc[:], 1.0)

        def warmup(n):
            # rotate over all 6 tail slots so the WAW chain never paces the
            # warmup below the engine rate
            for _ in range(n):
                wp = t_slot(512)
                nc.tensor.matmul(wp, wsc[:, 0:128], wsc[:], start=True, stop=True)

        # prologue DMAs (sync queue order = arrival order): hp0 weight
        # halves + K s0 + Q s0 form the minimal first-exp prefix
        w_load(wk_sb, wkP_d, 0)
        w_load(wq_sb, wqP_d, 0)
        first_load(KT_d, kin, "k")
        first_load(QT_d, qin, "q")
        nc.sync.dma_start(
            bq_sb.rearrange("p (hp c) -> p hp c", hp=2),
            bq_d.rearrange("(hp p) c -> p hp c", p=128))
        chunk_load(KT_d, kin, "k", 1)
        nc.sync.dma_start(id_sb[:], id_d[:, :])
        w_load(wv_sb, wvP_d)
        w_load(wk_sb, wkP_d, 1)
        w_load(wq_sb, wqP_d, 1)
        qk_proj("k", wk_sb, kT_sb, 0, 0, False)
        qk_proj("k", wk_sb, kT_sb, 0, 1, False)
        qk_proj("q", wq_sb, qT_sb, 0, 0, True)
        qk_proj("q", wq_sb, qT_sb, 0, 1, True)

        at(2, chunk_load, KT_d, kin, "k", 2)
        at(2, qk_proj, "k", wk_sb, kT_sb, 0, 2, False)
        at(3, chunk_load, KT_d, kin, "k", 3)
        at(3, qk_proj, "k", wk_sb, kT_sb, 0, 3, False)
        at(4, chunk_load, VT_d, vin, "v", 0)
        at(5, chunk_load, VT_d, vin, "v", 1)
        at(5, qk_proj, "k", wk_sb, kT_sb, 0, 4, False)
        at(6, qk_proj, "k", wk_sb, kT_sb, 0, 5, False)
        at(6, chunk_load, VT_d, vin, "v", 2)
        at(7, qk_proj, "q", wq_sb, qT_sb, 1, 0, True)
        at(7, chunk_load, VT_d, vin, "v", 3)
        at(8, qk_proj, "k", wk_sb, kT_sb, 0, 6, False)
        at(9, qk_proj, "k", wk_sb, kT_sb, 0, 7, False)
        at(9, chunk_load, QT_d, qin, "q", 1)
        at(10, qk_proj, "q", wq_sb, qT_sb, 1, 1, True)
        at(10, wo_load)
        at(11, chunk_load, QT_d, qin, "q", 2)
        at(11, qk_proj, "k", wk_sb, kT_sb, 1, 0, False)
        at(12, qk_proj, "k", wk_sb, kT_sb, 1, 1, False)
        at(44, chunk_load, QT_d, qin, "q", 3)
        at(13, qk_proj, "k", wk_sb, kT_sb, 1, 2, False)
        at(14, qk_proj, "k", wk_sb, kT_sb, 1, 3, False)
        at(15, qk_proj, "k", wk_sb, kT_sb, 1, 4, False)
        at(18, qk_proj, "k", wk_sb, kT_sb, 1, 5, False)
        at(21, qk_proj, "k", wk_sb, kT_sb, 1, 6, False)
        at(24, qk_proj, "k", wk_sb, kT_sb, 1, 7, False)
        at(26, qk_proj, "q", wq_sb, qT_sb, 0, 2, True)
        at(28, qk_proj, "q", wq_sb, qT_sb, 0, 3, True)
        at(40, qk_proj, "q", wq_sb, qT_sb, 1, 2, True)
        at(42, qk_proj, "q", wq_sb, qT_sb, 1, 3, True)
        at(56, qk_proj, "q", wq_sb, qT_sb, 0, 4, True)
        at(58, qk_proj, "q", wq_sb, qT_sb, 0, 5, True)
        at(72, qk_proj, "q", wq_sb, qT_sb, 1, 4, True)
        at(74, qk_proj, "q", wq_sb, qT_sb, 1, 5, True)
        at(88, qk_proj, "q", wq_sb, qT_sb, 0, 6, True)
        at(90, qk_proj, "q", wq_sb, qT_sb, 0, 7, True)
        at(104, qk_proj, "q", wq_sb, qT_sb, 1, 6, True)
        at(106, qk_proj, "q", wq_sb, qT_sb, 1, 7, True)
        # out-proj spread through the back half to keep the PE backlog alive
        # (first use must follow the (qvb, hp1) sweep drain at ~32qvb+35)
        for qvb in range(3):
            for qs in range(4):
                at(48 + 26 * qvb + 6 * qs, out_proj, qvb, qs, False)

        j = 0  # ctx stream pointer into UNITS

        def ctx_lag(i):
            # sweep 0 trails 13..18 units (V still streaming; spreads the
            # vproj work), later sweeps 2; the 1.5-per-unit catch-up in the
            # emission loop decays the lag smoothly
            if i < NKC:
                return min(13 + i // 2, 18)
            return 1 if i >= NU - NKC else 2

        # within a unit: injections and trailing ctx work go BEFORE the
        # scores pair — the in-order PE can chew on them while waiting for
        # exp(u-2) to free the scores psum tag. Mid-stream drain work (hh1
        # norm + transposes) defers 2-items-per-unit so it never wedges the
        # scores stream.
        def final_drain():
            # last sweep: ctx c15, batched norm, transposes, then dc-major
            # out-proj over the 6-slot tail rotation
            qvb, hp, c = UNITS[NU - 1]
            ctx_mm(qvb, hp, c)
            norm(qvb, hp)
            for qs in range(4):
                transpose(qvb, hp, qs, tail=True)
            o_sbs = [osb.tile([128, D], bf16, name="o_sb") for _ in range(4)]
            for dc in range(2):
                for qs in range(4):
                    qc = (512 * qvb + 128 * qs, 512 * qvb + 128 * (qs + 1))
                    o_ps = t_slot(512)
                    for cn in range(2):
                        nc.tensor.matmul(
                            o_ps[:], ctxT_sb[cn][:, qc[0]:qc[1]],
                            wo_sb[cn][:, 512 * dc:512 * (dc + 1)],
                            start=(cn == 0), stop=(cn == 1))
                    dst = o_sbs[qs][:, 512 * dc:512 * (dc + 1)]
                    if (2 * dc + qs) % 2 == 0:
                        nc.scalar.copy(dst, o_ps[:])
                    else:
                        nc.vector.tensor_copy(dst, o_ps[:])
                    eng = nc.sync if (dc + qs) % 2 == 0 else nc.gpsimd
                    eng.dma_start(
                        out_d[qc[0]:qc[1], 512 * dc:512 * (dc + 1)], dst)

        deferred = []
        for u in range(NU + 24):
            for fn, a in pre[u]:
                fn(*a)
            for _ in range(2):
                if deferred:
                    deferred.pop(0)()
            emitted = 0
            while j < NU and j <= u - ctx_lag(j) and emitted < 2:
                qv2, hp2, c2 = UNITS[j]
                if j == NU - 1:
                    final_drain()
                    j += 1
                    break
                if j < NKC:
                    v_proj(c2)
                ctx_mm(qv2, hp2, c2)
                emitted += 1
                j += 1
                if c2 == NKC - 1:
                    norm_hh(qv2, hp2, 0)
                    deferred.append(
                        lambda q=qv2, h=hp2: norm_hh(q, h, 1))
                    for qs in range(4):
                        deferred.append(
                            lambda q=qv2, h=hp2, s=qs: transpose(q, h, s))
                    break  # don't cross a drain inside one unit
            if u < NU:
                scores_exp(*UNITS[u])
            if u >= NU and j >= NU:
                break

    nc.compile()
    return nc


def kernel(Q, K, V, wq, bq, wk, bk, wv, bv, wo, bo):
    import ml_dtypes
    from concourse.bass_utils import run_bass_kernel_spmd

    if "nc" not in _CACHE:
        _CACHE["nc"] = _build()
    nc = _CACHE["nc"]

    bf = ml_dtypes.bfloat16
    Q = np.asarray(Q, np.float32)
    K = np.asarray(K, np.float32)
    V = np.asarray(V, np.float32)
    QT = [np.ascontiguousarray(Q[b].T).astype(bf) for b in range(B)]
    KT = [np.ascontiguousarray(K[b].T).astype(bf) for b in range(B)]
    VT = [np.ascontiguousarray(V[b].T).astype(bf) for b in range(B)]
    def perm_qk(w, g):
        # [D, R] -> [128p, (hp, d, 128r)] with element [p,hp,d,r] =
        # wT[d*128+p, hp*128+r]
        wT = np.asarray(w, np.float32)[g * R:(g + 1) * R].T
        return np.ascontiguousarray(
            wT.reshape(8, 128, 2, 128).transpose(1, 2, 0, 3).reshape(128, 2048)
        ).astype(bf)

    def perm_v(w, g):
        wT = np.asarray(w, np.float32)[g * R:(g + 1) * R].T
        return np.ascontiguousarray(
            wT.reshape(8, 128, 256).transpose(1, 0, 2).reshape(128, 2048)
        ).astype(bf)

    wqP = [perm_qk(wq, g) for g in range(4)]
    wkP = [perm_qk(wk, g) for g in range(4)]
    wvP = [perm_v(wv, g) for g in range(4)]
    woT = [np.ascontiguousarray(np.asarray(wo, np.float32)[:, g * R:(g + 1) * R].T
                                ).astype(bf) for g in range(4)]
    bqs = [np.ascontiguousarray(np.asarray(bq, np.float32)[g * R:(g + 1) * R, None])
           for g in range(4)]
    ident = np.eye(128, dtype=np.float32)

    in_maps = []
    for c in range(NCORES):
        b, g = c // 4, c % 4
        in_maps.append({
            "QT": QT[b], "KT": KT[b], "VT": VT[b],
            "wqP": wqP[g], "wkP": wkP[g], "wvP": wvP[g], "woT": woT[g],
            "bq": bqs[g], "ident": ident,
        })

    global _LAST_IN_MAPS
    _LAST_IN_MAPS = in_maps
    res = run_bass_kernel_spmd(nc, in_maps, core_ids=list(range(NCORES)))

    host_bias = (np.asarray(bv, np.float32) @ np.asarray(wo, np.float32).T
                 + np.asarray(bo, np.float32))
    out = np.zeros((B, S, D), np.float32)
    for c in range(NCORES):
        out[c // 4] += np.asarray(res.results[c]["OUT"], np.float32)
    out += host_bias[None, None, :]
    return out


# revision 76
# speedup vs baseline: 1.0081x; 1.0081x over previous
"""MultiHeadAttention TRN2 kernel: B=2, S=2048, D=1024, H=16, DK=64, 8 cores.

Sharding: core c handles batch b=c//4 and heads hg=(c%4)*4 .. +3 (data + head
parallel). Projections are column-split by head; out-proj row-split; the
all-reduce after out-proj is done on host (sum of 4 partials per batch).

All activations/weights stream HBM<->SBUF as bf16 (host converts), halving
DMA on the serial DMA-engine resource. Matmul inputs are bf16 (1 cycle/row at
any moving size) except qT/kT which stay f32r for exp-input precision.

Device dataflow (per core):
  qT/kT = (w-slice).T @ QT/KT      -> [feat 128 (2 heads), seq] f32r, 256-wide
  v     = VT.T @ wv-slice          -> natural [kpos, 4*64] chunks -> v_all bf16
                                      [kpos, head*16*65] with ones col (den)
  scoresT[kpos,q] = kT-chunk.T @ qT  (K=64, both heads packed in one
                                      [128,1024] 2-bank PSUM tile)
  expT = exp(scoresT/8)            -> bf16 SBUF (ACT, the throughput floor)
  ctx[q, 65] += expT-slice.T @ v-chunk   (F=65 bf16, PSUM accum over kpos;
                                          col 64 accumulates the denominator)
  ctx_sb = ctx * recip(den)        -> [q, 128(2 heads)] f32 per qsub
  ctxT = PE-transpose(ctx_sb)      -> ctxT_sb [feat, q] bf16
  out[q, Dout] = ctxT.T @ wo       -> 256-wide chunks -> bf16 partial out

The whole thing is software-pipelined at DMA-chunk granularity: the hp0
weight halves + K s0 + Q s0 form a minimal first-exp prefix on the sync
queue (first exp at ~12us), then the remaining K/V/Q chunks stream in while
one global stream of 128 scores->exp units runs; ctx matmuls trail the exp
stream by 13..18 units during sweep 0 (V still arriving) then catch up to a
lag of 2; projections / out-proj / norms / transposes are injected between
units so the in-order PE never waits on data that hasn't arrived. The exp
stream runs back-to-back (1038 ns/tile) through the ACT-bound middle.

PSUM (8 banks): scA/scB [128,1024] x2 banks each (scores/exp dbuf) | ctxA,
ctxB [128,455] (7 of 8 per-sweep [128,65] accumulators, parity-alternating) |
ctxC [128,512] (8th accumulator per parity) | projC [128,512] (2 rotating
256-wide slots for q/k/v-proj, out-proj and ctx transposes).

Bias handling (exact): bq added on device (per-partition add in qT layout);
bk dropped (softmax shift-invariance); bv and bo folded on host as
out += bv @ wo.T + bo (softmax weights sum to 1).
"""

from contextlib import ExitStack

import numpy as np

B, S, D, H, DK = 2, 2048, 1024, 16, 64
NCORES = 8
HPC = H // (NCORES // B)      # heads per core = 4
R = HPC * DK                  # local feats = 256
NKC = S // 128                # 128-wide k chunks = 16
VW = 65                       # v chunk width (64 + ones col)

_CACHE = {}
_LAST_IN_MAPS = None


def _build():
    import concourse.mybir as mybir
    import concourse.tile as tile
    from concourse import bacc

    f32 = mybir.dt.float32
    f32r = mybir.dt.float32r
    bf16 = mybir.dt.bfloat16
    Exp = mybir.ActivationFunctionType.Exp
    Add = mybir.AluOpType.add
    Mult = mybir.AluOpType.mult

    nc = bacc.Bacc(
        "TRN2", target_bir_lowering=False, debug=False,
        enable_asserts=True, num_devices=NCORES,
    )

    QT_d = nc.dram_tensor("QT", [D, S], bf16, kind="ExternalInput").ap()
    KT_d = nc.dram_tensor("KT", [D, S], bf16, kind="ExternalInput").ap()
    VT_d = nc.dram_tensor("VT", [D, S], bf16, kind="ExternalInput").ap()
    # weights host-permuted to exact SBUF layout: wq/wk as [128, hp, d, 128]
    # (per-hp halves load separately, 2KB-contiguous rows), wv as [128, d, 256]
    wqP_d = nc.dram_tensor("wqP", [128, 2 * 8 * 128], bf16,
                           kind="ExternalInput").ap()
    wkP_d = nc.dram_tensor("wkP", [128, 2 * 8 * 128], bf16,
                           kind="ExternalInput").ap()
    wvP_d = nc.dram_tensor("wvP", [128, 8 * 256], bf16,
                           kind="ExternalInput").ap()
    woT_d = nc.dram_tensor("woT", [R, D], bf16, kind="ExternalInput").ap()
    bq_d = nc.dram_tensor("bq", [R, 1], f32, kind="ExternalInput").ap()
    id_d = nc.dram_tensor("ident", [128, 128], f32, kind="ExternalInput").ap()
    out_d = nc.dram_tensor("OUT", [S, D], bf16, kind="ExternalOutput").ap()

    with tile.TileContext(nc) as tc, ExitStack() as ctx:
        sb = ctx.enter_context(tc.tile_pool(name="sb", bufs=1))
        qin = ctx.enter_context(tc.tile_pool(name="qin", bufs=6))
        kin = ctx.enter_context(tc.tile_pool(name="kin", bufs=8))
        vin = ctx.enter_context(tc.tile_pool(name="vin", bufs=6))
        expp = ctx.enter_context(tc.tile_pool(name="expp", bufs=20))
        cxp = ctx.enter_context(tc.tile_pool(name="cxp", bufs=2))
        osb = ctx.enter_context(tc.tile_pool(name="osb", bufs=4))
        psum = ctx.enter_context(tc.tile_pool(name="psum", bufs=1, space="PSUM"))

        # ---- persistent PSUM containers (8 banks exactly) ----
        # PSUM accumulation groups are zero-region (= bank) granular: a
        # start_tensor_calc matmul zeroes its whole bank, so each bank holds
        # exactly one live group. Scores halves are full banks; the 4 ctx
        # accumulators of one hh live in one bank as a single group; proj /
        # out-proj / transpose rotate through two whole-bank slots.
        sc_ps = [psum.tile([128, 1024], f32, name=f"sc{i}") for i in range(2)]
        ctxH = [psum.tile([128, 260], f32, name=f"ctxh{i}") for i in range(2)]
        projAB = [psum.tile([128, 512], f32, name=f"proj{i}") for i in range(2)]

        cnt = {"p": 0}

        def p_slot(w):
            # rotating whole-bank psum slot for q/k/v-proj, out-proj and
            # transposes; overlapping-view hazards serialize reuse
            i = cnt["p"]; cnt["p"] += 1
            return projAB[i % 2][:, 0:w]

        # ---- persistent SBUF ----
        wq_sb = sb.tile([128, 8 * R], bf16)   # (hp, d) block at 1024*hp+128*d
        wk_sb = sb.tile([128, 8 * R], bf16)
        wv_sb = sb.tile([128, 8 * R], bf16)   # D-chunk d at cols [R*d : +R]
        wo_sb = [sb.tile([128, D], bf16, name=f"wo_sb{cn}") for cn in range(2)]
        bq_sb = sb.tile([128, 2], f32)
        id_sb = sb.tile([128, 128], f32)

        qT_sb = [sb.tile([128, S], bf16, name=f"qT_sb{hp}") for hp in range(2)]
        kT_sb = [sb.tile([128, S], bf16, name=f"kT_sb{hp}") for hp in range(2)]
        v_all = sb.tile([128, HPC * NKC * VW], bf16)  # (h, c) at (h*NKC+c)*VW
        ctxT_sb = [sb.tile([128, S], bf16, name=f"ctxT_sb{cn}") for cn in range(2)]

        onecol = sb.tile([128, 1], f32)
        nc.vector.memset(onecol[:], 1.0)
        vv = v_all.rearrange("p (n c) -> p n c", c=VW)[:, :, 64:65].rearrange(
            "p n c -> p (n c)")
        nc.vector.tensor_copy(vv, onecol[:].broadcast_to((128, HPC * NKC)))

        def w_load(w_sb, w_d, hp=None):
            if hp is None:
                nc.sync.dma_start(w_sb[:], w_d[:, :])
            else:
                nc.sync.dma_start(w_sb[:, 1024 * hp:1024 * (hp + 1)],
                                  w_d[:, 1024 * hp:1024 * (hp + 1)])

        # staging tiles: one [128, 2048] bf16 tile covers 4 d-chunks x 512
        # seq; a (tensor, sblk) pair = 2 tiles (d 0-3, d 4-7)
        stage = {}

        def chunk_load(src, pool, tag, sblk):
            tiles = []
            for hf in range(2):
                t = pool.tile([128, 2048], bf16, name=tag, tag=tag)
                nc.sync.dma_start(
                    t.rearrange("p (d s) -> p d s", d=4),
                    src.rearrange("(d p) s -> p d s", p=128)[
                        :, 4 * hf:4 * hf + 4, 512 * sblk:512 * (sblk + 1)])
                tiles.append(t)
            stage[(tag, sblk)] = ("std", tiles)

        def first_load(src, pool, tag):
            # s0 of K/Q as two all-d x 256-col chunks so the first proj
            # tile only waits for half the data
            tiles = []
            for half in range(2):
                t = pool.tile([128, 2048], bf16, name=tag, tag=tag)
                nc.sync.dma_start(
                    t.rearrange("p (d s) -> p d s", d=8),
                    src.rearrange("(d p) s -> p d s", p=128)[
                        :, :, 256 * half:256 * (half + 1)])
                tiles.append(t)
            stage[(tag, 0)] = ("first", tiles)

        def staged(tag, sblk, d, cols):
            # d-chunk d of sblk, column slice `cols` within the 512-wide sblk
            ent = stage[(tag, sblk)]
            if ent[0] == "first":
                half = cols[0] // 256
                off = cols[0] - 256 * half
                return ent[1][half][
                    :, 256 * d + off:256 * d + off + cols[1] - cols[0]]
            t = ent[1][d // 4]
            base = 512 * (d % 4)
            return t[:, base + cols[0]:base + cols[1]]

        # ---- projection tiles ----
        def qk_proj(tag, w_sb, dst_sb, hp, j, bias):
            # (hp, j): 256 seq cols [256j : 256j+256] of head-pair hp
            sblk, half = j // 2, j % 2
            cols = (256 * half, 256 * half + 256)
            p_ps = p_slot(256)
            for d in range(8):
                nc.tensor.matmul(
                    p_ps[:],
                    w_sb[:, 1024 * hp + 128 * d:1024 * hp + 128 * (d + 1)],
                    staged(tag, sblk, d, cols), start=(d == 0), stop=(d == 7))
            dst = dst_sb[hp][:, 256 * j:256 * (j + 1)]
            if bias:
                nc.vector.tensor_scalar(
                    dst, p_ps[:], bq_sb[:, hp:hp + 1], None, op0=Add)
            else:
                nc.vector.tensor_copy(dst, p_ps[:])

        def v_proj(c):
            # kpos chunk c (128 rows): out [kpos, 256 feats] -> v_all slices
            sblk, sub = c // 4, c % 4
            cols = (128 * sub, 128 * sub + 128)
            v_ps = p_slot(256)
            for d in range(8):
                nc.tensor.matmul(
                    v_ps[:], staged("v", sblk, d, cols),
                    wv_sb[:, R * d:R * (d + 1)], start=(d == 0), stop=(d == 7))
            va = v_all.rearrange("p (h n c) -> p h n c", h=HPC, n=NKC)
            nc.vector.tensor_copy(
                va[:, :, c:c + 1, 0:64],
                v_ps[:].rearrange("p (h n c) -> p h n c", h=HPC, n=1))

        # ---- attention sweep pieces (scores/exp stream + trailing ctx) ----
        exp_ring = {}

        def scores_exp(qvb, hp, c):
            s_ps = sc_ps[c % 2]
            for hh in range(2):
                nc.tensor.matmul(
                    s_ps[:, 512 * hh:512 * (hh + 1)],
                    kT_sb[hp][64 * hh:64 * (hh + 1), 128 * c:128 * (c + 1)],
                    qT_sb[hp][64 * hh:64 * (hh + 1), 512 * qvb:512 * (qvb + 1)],
                    start=True, stop=True)
            expT = expp.tile([128, 1024], bf16, name="expT")
            nc.scalar.activation(expT[:], s_ps[:], Exp, scale=0.125)
            exp_ring[(qvb, hp, c)] = expT

        def ctx_mm(qvb, hp, c):
            expT = exp_ring.pop((qvb, hp, c))
            for hh in range(2):
                gh = HPC // 2 * hp + hh
                for qs in range(4):
                    nc.tensor.matmul(
                        ctxH[hh][0:128, 65 * qs:65 * qs + VW],
                        expT[:, 512 * hh + 128 * qs:512 * hh + 128 * (qs + 1)],
                        v_all[:, (gh * NKC + c) * VW:(gh * NKC + c + 1) * VW],
                        start=(c == 0 and qs == 0),
                        stop=(c == NKC - 1 and qs == 3))

        ctx_stage = {}

        def norm_hh(qvb, hp, hh):
            # drain one ctxH bank: a strided recip over the 4 denominator
            # columns + one strided multiply into the (qs, hh, 64) staging
            # tile cs
            if hh == 0:
                ctx_stage[(qvb, hp)] = cxp.tile(
                    [128, 512], f32, name="ctxs", tag=f"ctxs{hp}")
            cs = ctx_stage[(qvb, hp)]
            t3 = ctxH[hh].rearrange("p (qs w) -> p qs w", w=VW)
            rb = cxp.tile([128, 4], f32, name="rb", tag=f"rb{hh}")
            nc.vector.reciprocal_approx_fast(
                out=rb[:], in_=t3[:, :, 64:65].rearrange("p a b -> p (a b)"))
            nc.vector.tensor_mul(
                cs.rearrange("p (qs hh f) -> p qs hh f", qs=4, hh=2)[
                    :, :, hh, :],
                t3[:, :, 0:64],
                rb.rearrange("p (a b) -> p a b", b=1).broadcast_to(
                    (128, 4, 64)))

        def norm(qvb, hp):
            norm_hh(qvb, hp, 0)
            norm_hh(qvb, hp, 1)

        def transpose(qvb, hp, qs, tail=False):
            cs = ctx_stage[(qvb, hp)]
            lhsT = cs[:, 128 * qs:128 * (qs + 1)]
            tp = t_slot(128) if tail else p_slot(128)
            nc.tensor.matmul(tp, lhsT, id_sb[:], is_transpose=True)
            # after the last exp ACT is idle; split evacs across ACT and DVE
            dst = ctxT_sb[hp][:, 512 * qvb + 128 * qs:512 * qvb + 128 * (qs + 1)]
            if tail and qs % 2 == 0:
                nc.scalar.copy(dst, tp)
            else:
                nc.vector.tensor_copy(dst, tp)
            if qs == 3:
                del ctx_stage[(qvb, hp)]

        # after the final exp the score banks are free: the tail out-proj /
        # transposes rotate over 4 whole-bank psum slots instead of 2
        tail_slots = [lambda w: projAB[0][:, 0:w], lambda w: projAB[1][:, 0:w],
                      lambda w: sc_ps[0][:, 0:w], lambda w: sc_ps[0][:, 512:512 + w],
                      lambda w: sc_ps[1][:, 0:w], lambda w: sc_ps[1][:, 512:512 + w]]

        def t_slot(w):
            i = cnt["p"]; cnt["p"] += 1
            return tail_slots[i % 6](w)

        def out_proj(qvb, qs, tail=False):
            o_sb = osb.tile([128, D], bf16, name="o_sb")
            qcols = (512 * qvb + 128 * qs, 512 * qvb + 128 * (qs + 1))
            for dc in range(2):
                o_ps = t_slot(512) if tail else p_slot(512)
                for cn in range(2):
                    nc.tensor.matmul(
                        o_ps[:], ctxT_sb[cn][:, qcols[0]:qcols[1]],
                        wo_sb[cn][:, 512 * dc:512 * (dc + 1)],
                        start=(cn == 0), stop=(cn == 1))
                if tail and dc % 2 == 0:
                    nc.scalar.copy(o_sb[:, 512 * dc:512 * (dc + 1)], o_ps[:])
                else:
                    nc.vector.tensor_copy(o_sb[:, 512 * dc:512 * (dc + 1)],
                                          o_ps[:])
                # half-store right after its evac, alternating queues so the
                # final stores drain two DGE pipelines in parallel; the last
                # store goes on sync (no gpsimd Q7 launch on the tail)
                eng = nc.gpsimd if dc == 0 else nc.sync
                eng.dma_start(
                    out_d[qcols[0]:qcols[1], 512 * dc:512 * (dc + 1)],
                    o_sb[:, 512 * dc:512 * (dc + 1)])

        # ================= emission schedule =================
        # One global stream of 128 exp units ((qvb, hp) sweeps, kchunk
        # minor). ctx matmuls trail by 13 units during sweep 0 (V still
        # streaming), then catch up 2-per-unit to a lag of 2 so the tail
        # stays short.
        UNITS = [(s // 2, s % 2, c) for s in range(8) for c in range(NKC)]
        NU = len(UNITS)
        pre = {u: [] for u in range(NU + 16)}

        def at(u, fn, *a):
            pre[u].append((fn, a))

        def wo_load():
            for cn in range(2):
                nc.sync.dma_start(wo_sb[cn][:], woT_d[128 * cn:128 * (cn + 1), :])

        # PE warmup: keep the tensor engine busy (and its p-state ramp hot)
        # through the DMA-bound prologue; calibrated to end near Ks0 arrival
        wsc = sb.tile([128, 512], bf16)
        nc.vector.memset(wsc[:], 1.0)

        def warmup(n):
            # rotate over all 6 tail slots so the WAW chain never paces the
            # warmup below the engine rate
            for _ in range(n):
                wp = t_slot(512)
                nc.tensor.matmul(wp, wsc[:, 0:128], wsc[:], start=True, stop=True)

        # prologue DMAs (sync queue order = arrival order): hp0 weight
        # halves + K s0 + Q s0 form the minimal first-exp prefix
        w_load(wk_sb, wkP_d, 0)
        w_load(wq_sb, wqP_d, 0)
        first_load(KT_d, kin, "k")
        first_load(QT_d, qin, "q")
        nc.sync.dma_start(
            bq_sb.rearrange("p (hp c) -> p hp c", hp=2),
            bq_d.rearrange("(hp p) c -> p hp c", p=128))
        chunk_load(KT_d, kin, "k", 1)
        nc.sync.dma_start(id_sb[:], id_d[:, :])
        w_load(wv_sb, wvP_d)
        w_load(wk_sb, wkP_d, 1)
        w_load(wq_sb, wqP_d, 1)
        qk_proj("k", wk_sb, kT_sb, 0, 0, False)
        qk_proj("k", wk_sb, kT_sb, 0, 1, False)
        qk_proj("q", wq_sb, qT_sb, 0, 0, True)
        qk_proj("q", wq_sb, qT_sb, 0, 1, True)

        at(2, chunk_load, KT_d, kin, "k", 2)
        at(2, qk_proj, "k", wk_sb, kT_sb, 0, 2, False)
        at(3, chunk_load, KT_d, kin, "k", 3)
        at(3, qk_proj, "k", wk_sb, kT_sb, 0, 3, False)
        at(4, chunk_load, VT_d, vin, "v", 0)
        at(5, chunk_load, VT_d, vin, "v", 1)
        at(5, qk_proj, "k", wk_sb, kT_sb, 0, 4, False)
        at(6, qk_proj, "k", wk_sb, kT_sb, 0, 5, False)
        at(6, chunk_load, VT_d, vin, "v", 2)
        at(7, qk_proj, "q", wq_sb, qT_sb, 1, 0, True)
        at(7, chunk_load, VT_d, vin, "v", 3)
        at(8, qk_proj, "k", wk_sb, kT_sb, 0, 6, False)
        at(9, qk_proj, "k", wk_sb, kT_sb, 0, 7, False)
        at(9, chunk_load, QT_d, qin, "q", 1)
        at(10, qk_proj, "q", wq_sb, qT_sb, 1, 1, True)
        at(10, wo_load)
        at(11, chunk_load, QT_d, qin, "q", 2)
        at(11, qk_proj, "k", wk_sb, kT_sb, 1, 0, False)
        at(12, qk_proj, "k", wk_sb, kT_sb, 1, 1, False)
        at(44, chunk_load, QT_d, qin, "q", 3)
        at(13, qk_proj, "k", wk_sb, kT_sb, 1, 2, False)
        at(14, qk_proj, "k", wk_sb, kT_sb, 1, 3, False)
        at(15, qk_proj, "k", wk_sb, kT_sb, 1, 4, False)
        at(18, qk_proj, "k", wk_sb, kT_sb, 1, 5, False)
        at(21, qk_proj, "k", wk_sb, kT_sb, 1, 6, False)
        at(24, qk_proj, "k", wk_sb, kT_sb, 1, 7, False)
        at(26, qk_proj, "q", wq_sb, qT_sb, 0, 2, True)
        at(28, qk_proj, "q", wq_sb, qT_sb, 0, 3, True)
        at(40, qk_proj, "q", wq_sb, qT_sb, 1, 2, True)
        at(42, qk_proj, "q", wq_sb, qT_sb, 1, 3, True)
        at(56, qk_proj, "q", wq_sb, qT_sb, 0, 4, True)
        at(58, qk_proj, "q", wq_sb, qT_sb, 0, 5, True)
        at(72, qk_proj, "q", wq_sb, qT_sb, 1, 4, True)
        at(74, qk_proj, "q", wq_sb, qT_sb, 1, 5, True)
        at(88, qk_proj, "q", wq_sb, qT_sb, 0, 6, True)
        at(90, qk_proj, "q", wq_sb, qT_sb, 0, 7, True)
        at(104, qk_proj, "q", wq_sb, qT_sb, 1, 6, True)
        at(106, qk_proj, "q", wq_sb, qT_sb, 1, 7, True)
        # out-proj spread through the back half to keep the PE backlog alive
        # (first use must follow the (qvb, hp1) sweep drain at ~32qvb+35)
        for qvb in range(3):
            for qs in range(4):
                at(48 + 26 * qvb + 6 * qs, out_proj, qvb, qs, False)

        j = 0  # ctx stream pointer into UNITS

        def ctx_lag(i):
            # sweep 0 trails 13..18 units (V still streaming; spreads the
            # vproj work), later sweeps 2; the 1.5-per-unit catch-up in the
            # emission loop decays the lag smoothly
            if i < NKC:
                return min(13 + i // 2, 18)
            return 1 if i >= NU - NKC else 2

        # within a unit: injections and trailing ctx work go BEFORE the
        # scores pair — the in-order PE can chew on them while waiting for
        # exp(u-2) to free the scores psum tag. Mid-stream drain work (hh1
        # norm + transposes) defers 2-items-per-unit so it never wedges the
        # scores stream.
        def final_drain():
            # last sweep: ctx c15, batched norm, transposes, then dc-major
            # out-proj over the 6-slot tail rotation
            qvb, hp, c = UNITS[NU - 1]
            ctx_mm(qvb, hp, c)
            norm(qvb, hp)
            for qs in range(4):
                transpose(qvb, hp, qs, tail=True)
            o_sbs = [osb.tile([128, D], bf16, name="o_sb") for _ in range(4)]
            for dc in range(2):
                for qs in range(4):
                    qc = (512 * qvb + 128 * qs, 512 * qvb + 128 * (qs + 1))
                    o_ps = t_slot(512)
                    for cn in range(2):
                        nc.tensor.matmul(
                            o_ps[:], ctxT_sb[cn][:, qc[0]:qc[1]],
                            wo_sb[cn][:, 512 * dc:512 * (dc + 1)],
                            start=(cn == 0), stop=(cn == 1))
                    dst = o_sbs[qs][:, 512 * dc:512 * (dc + 1)]
                    if (2 * dc + qs) % 2 == 0:
                        nc.scalar.copy(dst, o_ps[:])
                    else:
                        nc.vector.tensor_copy(dst, o_ps[:])
                    eng = nc.sync if (dc + qs) % 2 == 0 else nc.gpsimd
                    eng.dma_start(
                        out_d[qc[0]:qc[1], 512 * dc:512 * (dc + 1)], dst)

        deferred = []
        for u in range(NU + 24):
            for fn, a in pre[u]:
                fn(*a)
            for _ in range(2):
                if deferred:
                    deferred.pop(0)()
            emitted = 0
            cap = 2 if u % 2 else 1
            while j < NU and j <= u - ctx_lag(j) and emitted < cap:
                qv2, hp2, c2 = UNITS[j]
                if j == NU - 1:
                    final_drain()
                    j += 1
                    break
                if j < NKC:
                    v_proj(c2)
                ctx_mm(qv2, hp2, c2)
                emitted += 1
                j += 1
                if c2 == NKC - 1:
                    norm_hh(qv2, hp2, 0)
                    deferred.append(
                        lambda q=qv2, h=hp2: norm_hh(q, h, 1))
                    for qs in range(4):
                        deferred.append(
                            lambda q=qv2, h=hp2, s=qs: transpose(q, h, s))
                    break  # don't cross a drain inside one unit
            if u < NU:
                scores_exp(*UNITS[u])
            if u >= NU and j >= NU:
                break

    nc.compile()
    return nc


def kernel(Q, K, V, wq, bq, wk, bk, wv, bv, wo, bo):
    import ml_dtypes
    from concourse.bass_utils import run_bass_kernel_spmd

    if "nc" not in _CACHE:
        _CACHE["nc"] = _build()
    nc = _CACHE["nc"]

    bf = ml_dtypes.bfloat16
    Q = np.asarray(Q, np.float32)
    K = np.asarray(K, np.float32)
    V = np.asarray(V, np.float32)
    QT = [np.ascontiguousarray(Q[b].T).astype(bf) for b in range(B)]
    KT = [np.ascontiguousarray(K[b].T).astype(bf) for b in range(B)]
    VT = [np.ascontiguousarray(V[b].T).astype(bf) for b in range(B)]
    def perm_qk(w, g):
        # [D, R] -> [128p, (hp, d, 128r)] with element [p,hp,d,r] =
        # wT[d*128+p, hp*128+r]
        wT = np.asarray(w, np.float32)[g * R:(g + 1) * R].T
        return np.ascontiguousarray(
            wT.reshape(8, 128, 2, 128).transpose(1, 2, 0, 3).reshape(128, 2048)
        ).astype(bf)

    def perm_v(w, g):
        wT = np.asarray(w, np.float32)[g * R:(g + 1) * R].T
        return np.ascontiguousarray(
            wT.reshape(8, 128, 256).transpose(1, 0, 2).reshape(128, 2048)
        ).astype(bf)

    wqP = [perm_qk(wq, g) for g in range(4)]
    wkP = [perm_qk(wk, g) for g in range(4)]
    wvP = [perm_v(wv, g) for g in range(4)]
    woT = [np.ascontiguousarray(np.asarray(wo, np.float32)[:, g * R:(g + 1) * R].T
                                ).astype(bf) for g in range(4)]
    bqs = [np.ascontiguousarray(np.asarray(bq, np.float32)[g * R:(g + 1) * R, None])
           for g in range(4)]
    ident = np.eye(128, dtype=np.float32)

    in_maps = []
    for c in range(NCORES):
        b, g = c // 4, c % 4
        in_maps.append({
            "QT": QT[b], "KT": KT[b], "VT": VT[b],
            "wqP": wqP[g], "wkP": wkP[g], "wvP": wvP[g], "woT": woT[g],
            "bq": bqs[g], "ident": ident,
        })

    global _LAST_IN_MAPS
    _LAST_IN_MAPS = in_maps
    res = run_bass_kernel_spmd(nc, in_maps, core_ids=list(range(NCORES)))

    host_bias = (np.asarray(bv, np.float32) @ np.asarray(wo, np.float32).T
                 + np.asarray(bo, np.float32))
    out = np.zeros((B, S, D), np.float32)
    for c in range(NCORES):
        out[c // 4] += np.asarray(res.results[c]["OUT"], np.float32)
    out += host_bias[None, None, :]
    return out


# revision 78
# speedup vs baseline: 1.0120x; 1.0039x over previous
"""MultiHeadAttention TRN2 kernel: B=2, S=2048, D=1024, H=16, DK=64, 8 cores.

Sharding: core c handles batch b=c//4 and heads hg=(c%4)*4 .. +3 (data + head
parallel). Projections are column-split by head; out-proj row-split; the
all-reduce after out-proj is done on host (sum of 4 partials per batch).

All activations/weights stream HBM<->SBUF as bf16 (host converts), halving
DMA on the serial DMA-engine resource. Matmul inputs are bf16 (1 cycle/row at
any moving size) except qT/kT which stay f32r for exp-input precision.

Device dataflow (per core):
  qT/kT = (w-slice).T @ QT/KT      -> [feat 128 (2 heads), seq] f32r, 256-wide
  v     = VT.T @ wv-slice          -> natural [kpos, 4*64] chunks -> v_all bf16
                                      [kpos, head*16*65] with ones col (den)
  scoresT[kpos,q] = kT-chunk.T @ qT  (K=64, both heads packed in one
                                      [128,1024] 2-bank PSUM tile)
  expT = exp(scoresT/8)            -> bf16 SBUF (ACT, the throughput floor)
  ctx[q, 65] += expT-slice.T @ v-chunk   (F=65 bf16, PSUM accum over kpos;
                                          col 64 accumulates the denominator)
  ctx_sb = ctx * recip(den)        -> [q, 128(2 heads)] f32 per qsub
  ctxT = PE-transpose(ctx_sb)      -> ctxT_sb [feat, q] bf16
  out[q, Dout] = ctxT.T @ wo       -> 256-wide chunks -> bf16 partial out

The whole thing is software-pipelined at DMA-chunk granularity: the hp0
weight halves + K s0 + Q s0 form a minimal first-exp prefix on the sync
queue (first exp at ~12us), then the remaining K/V/Q chunks stream in while
one global stream of 128 scores->exp units runs; ctx matmuls trail the exp
stream by 13..18 units during sweep 0 (V still arriving) then catch up to a
lag of 2; projections / out-proj / norms / transposes are injected between
units so the in-order PE never waits on data that hasn't arrived. The exp
stream runs back-to-back (1038 ns/tile) through the ACT-bound middle.

PSUM (8 banks): scA/scB [128,1024] x2 banks each (scores/exp dbuf) | ctxA,
ctxB [128,455] (7 of 8 per-sweep [128,65] accumulators, parity-alternating) |
ctxC [128,512] (8th accumulator per parity) | projC [128,512] (2 rotating
256-wide slots for q/k/v-proj, out-proj and ctx transposes).

Bias handling (exact): bq added on device (per-partition add in qT layout);
bk dropped (softmax shift-invariance); bv and bo folded on host as
out += bv @ wo.T + bo (softmax weights sum to 1).
"""

from contextlib import ExitStack

import numpy as np

B, S, D, H, DK = 2, 2048, 1024, 16, 64
NCORES = 8
HPC = H // (NCORES // B)      # heads per core = 4
R = HPC * DK                  # local feats = 256
NKC = S // 128                # 128-wide k chunks = 16
VW = 65                       # v chunk width (64 + ones col)

_CACHE = {}
_LAST_IN_MAPS = None


def _build():
    import concourse.mybir as mybir
    import concourse.tile as tile
    from concourse import bacc

    f32 = mybir.dt.float32
    f32r = mybir.dt.float32r
    bf16 = mybir.dt.bfloat16
    Exp = mybir.ActivationFunctionType.Exp
    Add = mybir.AluOpType.add
    Mult = mybir.AluOpType.mult

    nc = bacc.Bacc(
        "TRN2", target_bir_lowering=False, debug=False,
        enable_asserts=True, num_devices=NCORES,
    )

    QT_d = nc.dram_tensor("QT", [D, S], bf16, kind="ExternalInput").ap()
    KT_d = nc.dram_tensor("KT", [D, S], bf16, kind="ExternalInput").ap()
    VT_d = nc.dram_tensor("VT", [D, S], bf16, kind="ExternalInput").ap()
    # weights host-permuted to exact SBUF layout: wq/wk as [128, hp, d, 128]
    # (per-hp halves load separately, 2KB-contiguous rows), wv as [128, d, 256]
    wqP_d = nc.dram_tensor("wqP", [128, 2 * 8 * 128], bf16,
                           kind="ExternalInput").ap()
    wkP_d = nc.dram_tensor("wkP", [128, 2 * 8 * 128], bf16,
                           kind="ExternalInput").ap()
    wvP_d = nc.dram_tensor("wvP", [128, 8 * 256], bf16,
                           kind="ExternalInput").ap()
    woT_d = nc.dram_tensor("woT", [R, D], bf16, kind="ExternalInput").ap()
    bq_d = nc.dram_tensor("bq", [R, 1], f32, kind="ExternalInput").ap()
    id_d = nc.dram_tensor("ident", [128, 128], f32, kind="ExternalInput").ap()
    out_d = nc.dram_tensor("OUT", [S, D], bf16, kind="ExternalOutput").ap()

    with tile.TileContext(nc) as tc, ExitStack() as ctx:
        sb = ctx.enter_context(tc.tile_pool(name="sb", bufs=1))
        qin = ctx.enter_context(tc.tile_pool(name="qin", bufs=6))
        kin = ctx.enter_context(tc.tile_pool(name="kin", bufs=8))
        vin = ctx.enter_context(tc.tile_pool(name="vin", bufs=6))
        expp = ctx.enter_context(tc.tile_pool(name="expp", bufs=23))
        cxp = ctx.enter_context(tc.tile_pool(name="cxp", bufs=2))
        osb = ctx.enter_context(tc.tile_pool(name="osb", bufs=4))
        psum = ctx.enter_context(tc.tile_pool(name="psum", bufs=1, space="PSUM"))

        # ---- persistent PSUM containers (8 banks exactly) ----
        # PSUM accumulation groups are zero-region (= bank) granular: a
        # start_tensor_calc matmul zeroes its whole bank, so each bank holds
        # exactly one live group. Scores halves are full banks; the 4 ctx
        # accumulators of one hh live in one bank as a single group; proj /
        # out-proj / transpose rotate through two whole-bank slots.
        sc_ps = [psum.tile([128, 1024], f32, name=f"sc{i}") for i in range(2)]
        ctxH = [psum.tile([128, 260], f32, name=f"ctxh{i}") for i in range(2)]
        projAB = [psum.tile([128, 512], f32, name=f"proj{i}") for i in range(2)]

        cnt = {"p": 0}

        def p_slot(w):
            # rotating whole-bank psum slot for q/k/v-proj, out-proj and
            # transposes; overlapping-view hazards serialize reuse
            i = cnt["p"]; cnt["p"] += 1
            return projAB[i % 2][:, 0:w]

        # ---- persistent SBUF ----
        wq_sb = sb.tile([128, 8 * R], bf16)   # (hp, d) block at 1024*hp+128*d
        wk_sb = sb.tile([128, 8 * R], bf16)
        wv_sb = sb.tile([128, 8 * R], bf16)   # D-chunk d at cols [R*d : +R]
        wo_sb = [sb.tile([128, D], bf16, name=f"wo_sb{cn}") for cn in range(2)]
        bq_sb = sb.tile([128, 2], f32)
        id_sb = sb.tile([128, 128], f32)

        qT_sb = [sb.tile([128, S], bf16, name=f"qT_sb{hp}") for hp in range(2)]
        kT_sb = [sb.tile([128, S], bf16, name=f"kT_sb{hp}") for hp in range(2)]
        v_all = sb.tile([128, HPC * NKC * VW], bf16)  # (h, c) at (h*NKC+c)*VW
        ctxT_sb = [sb.tile([128, S], bf16, name=f"ctxT_sb{cn}") for cn in range(2)]

        onecol = sb.tile([128, 1], f32)
        nc.vector.memset(onecol[:], 1.0)
        vv = v_all.rearrange("p (n c) -> p n c", c=VW)[:, :, 64:65].rearrange(
            "p n c -> p (n c)")
        nc.vector.tensor_copy(vv, onecol[:].broadcast_to((128, HPC * NKC)))

        def w_load(w_sb, w_d, hp=None):
            if hp is None:
                nc.sync.dma_start(w_sb[:], w_d[:, :])
            else:
                nc.sync.dma_start(w_sb[:, 1024 * hp:1024 * (hp + 1)],
                                  w_d[:, 1024 * hp:1024 * (hp + 1)])

        # staging tiles: one [128, 2048] bf16 tile covers 4 d-chunks x 512
        # seq; a (tensor, sblk) pair = 2 tiles (d 0-3, d 4-7)
        stage = {}

        def chunk_load(src, pool, tag, sblk):
            tiles = []
            for hf in range(2):
                t = pool.tile([128, 2048], bf16, name=tag, tag=tag)
                nc.sync.dma_start(
                    t.rearrange("p (d s) -> p d s", d=4),
                    src.rearrange("(d p) s -> p d s", p=128)[
                        :, 4 * hf:4 * hf + 4, 512 * sblk:512 * (sblk + 1)])
                tiles.append(t)
            stage[(tag, sblk)] = ("std", tiles)

        def first_load(src, pool, tag):
            # s0 of K/Q as two all-d x 256-col chunks so the first proj
            # tile only waits for half the data
            tiles = []
            for half in range(2):
                t = pool.tile([128, 2048], bf16, name=tag, tag=tag)
                nc.sync.dma_start(
                    t.rearrange("p (d s) -> p d s", d=8),
                    src.rearrange("(d p) s -> p d s", p=128)[
                        :, :, 256 * half:256 * (half + 1)])
                tiles.append(t)
            stage[(tag, 0)] = ("first", tiles)

        def staged(tag, sblk, d, cols):
            # d-chunk d of sblk, column slice `cols` within the 512-wide sblk
            ent = stage[(tag, sblk)]
            if ent[0] == "first":
                half = cols[0] // 256
                off = cols[0] - 256 * half
                return ent[1][half][
                    :, 256 * d + off:256 * d + off + cols[1] - cols[0]]
            t = ent[1][d // 4]
            base = 512 * (d % 4)
            return t[:, base + cols[0]:base + cols[1]]

        # ---- projection tiles ----
        def qk_proj(tag, w_sb, dst_sb, hp, j, bias):
            # (hp, j): 256 seq cols [256j : 256j+256] of head-pair hp
            sblk, half = j // 2, j % 2
            cols = (256 * half, 256 * half + 256)
            p_ps = p_slot(256)
            for d in range(8):
                nc.tensor.matmul(
                    p_ps[:],
                    w_sb[:, 1024 * hp + 128 * d:1024 * hp + 128 * (d + 1)],
                    staged(tag, sblk, d, cols), start=(d == 0), stop=(d == 7))
            dst = dst_sb[hp][:, 256 * j:256 * (j + 1)]
            if bias:
                nc.vector.tensor_scalar(
                    dst, p_ps[:], bq_sb[:, hp:hp + 1], None, op0=Add)
            else:
                nc.vector.tensor_copy(dst, p_ps[:])

        def v_proj(c):
            # kpos chunk c (128 rows): out [kpos, 256 feats] -> v_all slices
            sblk, sub = c // 4, c % 4
            cols = (128 * sub, 128 * sub + 128)
            v_ps = p_slot(256)
            for d in range(8):
                nc.tensor.matmul(
                    v_ps[:], staged("v", sblk, d, cols),
                    wv_sb[:, R * d:R * (d + 1)], start=(d == 0), stop=(d == 7))
            va = v_all.rearrange("p (h n c) -> p h n c", h=HPC, n=NKC)
            nc.vector.tensor_copy(
                va[:, :, c:c + 1, 0:64],
                v_ps[:].rearrange("p (h n c) -> p h n c", h=HPC, n=1))

        # ---- attention sweep pieces (scores/exp stream + trailing ctx) ----
        exp_ring = {}

        def scores_exp(qvb, hp, c):
            s_ps = sc_ps[c % 2]
            for hh in range(2):
                nc.tensor.matmul(
                    s_ps[:, 512 * hh:512 * (hh + 1)],
                    kT_sb[hp][64 * hh:64 * (hh + 1), 128 * c:128 * (c + 1)],
                    qT_sb[hp][64 * hh:64 * (hh + 1), 512 * qvb:512 * (qvb + 1)],
                    start=True, stop=True)
            expT = expp.tile([128, 1024], bf16, name="expT")
            nc.scalar.activation(expT[:], s_ps[:], Exp, scale=0.125)
            exp_ring[(qvb, hp, c)] = expT

        def ctx_mm(qvb, hp, c):
            expT = exp_ring.pop((qvb, hp, c))
            for hh in range(2):
                gh = HPC // 2 * hp + hh
                for qs in range(4):
                    nc.tensor.matmul(
                        ctxH[hh][0:128, 65 * qs:65 * qs + VW],
                        expT[:, 512 * hh + 128 * qs:512 * hh + 128 * (qs + 1)],
                        v_all[:, (gh * NKC + c) * VW:(gh * NKC + c + 1) * VW],
                        start=(c == 0 and qs == 0),
                        stop=(c == NKC - 1 and qs == 3))

        ctx_stage = {}

        def norm_hh(qvb, hp, hh):
            # drain one ctxH bank: a strided recip over the 4 denominator
            # columns + one strided multiply into the (qs, hh, 64) staging
            # tile cs
            if hh == 0:
                ctx_stage[(qvb, hp)] = cxp.tile(
                    [128, 512], f32, name="ctxs", tag=f"ctxs{hp}")
            cs = ctx_stage[(qvb, hp)]
            t3 = ctxH[hh].rearrange("p (qs w) -> p qs w", w=VW)
            rb = cxp.tile([128, 4], f32, name="rb", tag=f"rb{hh}")
            nc.vector.reciprocal_approx_fast(
                out=rb[:], in_=t3[:, :, 64:65].rearrange("p a b -> p (a b)"))
            nc.vector.tensor_mul(
                cs.rearrange("p (qs hh f) -> p qs hh f", qs=4, hh=2)[
                    :, :, hh, :],
                t3[:, :, 0:64],
                rb.rearrange("p (a b) -> p a b", b=1).broadcast_to(
                    (128, 4, 64)))

        def norm(qvb, hp):
            norm_hh(qvb, hp, 0)
            norm_hh(qvb, hp, 1)

        def transpose(qvb, hp, qs, tail=False):
            cs = ctx_stage[(qvb, hp)]
            lhsT = cs[:, 128 * qs:128 * (qs + 1)]
            tp = t_slot(128) if tail else p_slot(128)
            nc.tensor.matmul(tp, lhsT, id_sb[:], is_transpose=True)
            # after the last exp ACT is idle; split evacs across ACT and DVE
            dst = ctxT_sb[hp][:, 512 * qvb + 128 * qs:512 * qvb + 128 * (qs + 1)]
            if tail and qs % 2 == 0:
                nc.scalar.copy(dst, tp)
            else:
                nc.vector.tensor_copy(dst, tp)
            if qs == 3:
                del ctx_stage[(qvb, hp)]

        # after the final exp the score banks are free: the tail out-proj /
        # transposes rotate over 4 whole-bank psum slots instead of 2
        tail_slots = [lambda w: projAB[0][:, 0:w], lambda w: projAB[1][:, 0:w],
                      lambda w: sc_ps[0][:, 0:w], lambda w: sc_ps[0][:, 512:512 + w],
                      lambda w: sc_ps[1][:, 0:w], lambda w: sc_ps[1][:, 512:512 + w]]

        def t_slot(w):
            i = cnt["p"]; cnt["p"] += 1
            return tail_slots[i % 6](w)

        def out_proj(qvb, qs, tail=False):
            o_sb = osb.tile([128, D], bf16, name="o_sb")
            qcols = (512 * qvb + 128 * qs, 512 * qvb + 128 * (qs + 1))
            for dc in range(2):
                o_ps = t_slot(512) if tail else p_slot(512)
                for cn in range(2):
                    nc.tensor.matmul(
                        o_ps[:], ctxT_sb[cn][:, qcols[0]:qcols[1]],
                        wo_sb[cn][:, 512 * dc:512 * (dc + 1)],
                        start=(cn == 0), stop=(cn == 1))
                if tail and dc % 2 == 0:
                    nc.scalar.copy(o_sb[:, 512 * dc:512 * (dc + 1)], o_ps[:])
                else:
                    nc.vector.tensor_copy(o_sb[:, 512 * dc:512 * (dc + 1)],
                                          o_ps[:])
                # half-store right after its evac, alternating queues so the
                # final stores drain two DGE pipelines in parallel; the last
                # store goes on sync (no gpsimd Q7 launch on the tail)
                eng = nc.gpsimd if dc == 0 else nc.sync
                eng.dma_start(
                    out_d[qcols[0]:qcols[1], 512 * dc:512 * (dc + 1)],
                    o_sb[:, 512 * dc:512 * (dc + 1)])

        # ================= emission schedule =================
        # One global stream of 128 exp units ((qvb, hp) sweeps, kchunk
        # minor). ctx matmuls trail by 13 units during sweep 0 (V still
        # streaming), then catch up 2-per-unit to a lag of 2 so the tail
        # stays short.
        UNITS = [(s // 2, s % 2, c) for s in range(8) for c in range(NKC)]
        NU = len(UNITS)
        pre = {u: [] for u in range(NU + 16)}

        def at(u, fn, *a):
            pre[u].append((fn, a))

        def wo_load():
            for cn in range(2):
                nc.sync.dma_start(wo_sb[cn][:], woT_d[128 * cn:128 * (cn + 1), :])

        # PE warmup: keep the tensor engine busy (and its p-state ramp hot)
        # through the DMA-bound prologue; calibrated to end near Ks0 arrival
        wsc = sb.tile([128, 512], bf16)
        nc.vector.memset(wsc[:], 1.0)

        def warmup(n):
            # rotate over all 6 tail slots so the WAW chain never paces the
            # warmup below the engine rate
            for _ in range(n):
                wp = t_slot(512)
                nc.tensor.matmul(wp, wsc[:, 0:128], wsc[:], start=True, stop=True)

        # prologue DMAs (sync queue order = arrival order): hp0 weight
        # halves + K s0 + Q s0 form the minimal first-exp prefix
        w_load(wk_sb, wkP_d, 0)
        w_load(wq_sb, wqP_d, 0)
        first_load(KT_d, kin, "k")
        first_load(QT_d, qin, "q")
        nc.sync.dma_start(
            bq_sb.rearrange("p (hp c) -> p hp c", hp=2),
            bq_d.rearrange("(hp p) c -> p hp c", p=128))
        chunk_load(KT_d, kin, "k", 1)
        nc.sync.dma_start(id_sb[:], id_d[:, :])
        w_load(wv_sb, wvP_d)
        w_load(wk_sb, wkP_d, 1)
        w_load(wq_sb, wqP_d, 1)
        qk_proj("k", wk_sb, kT_sb, 0, 0, False)
        qk_proj("k", wk_sb, kT_sb, 0, 1, False)
        qk_proj("q", wq_sb, qT_sb, 0, 0, True)
        qk_proj("q", wq_sb, qT_sb, 0, 1, True)

        at(2, chunk_load, KT_d, kin, "k", 2)
        at(2, qk_proj, "k", wk_sb, kT_sb, 0, 2, False)
        at(3, chunk_load, KT_d, kin, "k", 3)
        at(3, qk_proj, "k", wk_sb, kT_sb, 0, 3, False)
        at(4, chunk_load, VT_d, vin, "v", 0)
        at(5, chunk_load, VT_d, vin, "v", 1)
        at(5, qk_proj, "k", wk_sb, kT_sb, 0, 4, False)
        at(6, qk_proj, "k", wk_sb, kT_sb, 0, 5, False)
        at(6, chunk_load, VT_d, vin, "v", 2)
        at(7, qk_proj, "q", wq_sb, qT_sb, 1, 0, True)
        at(7, chunk_load, VT_d, vin, "v", 3)
        at(8, qk_proj, "k", wk_sb, kT_sb, 0, 6, False)
        at(9, qk_proj, "k", wk_sb, kT_sb, 0, 7, False)
        at(9, chunk_load, QT_d, qin, "q", 1)
        at(10, qk_proj, "q", wq_sb, qT_sb, 1, 1, True)
        at(10, wo_load)
        at(11, chunk_load, QT_d, qin, "q", 2)
        at(11, qk_proj, "k", wk_sb, kT_sb, 1, 0, False)
        at(12, qk_proj, "k", wk_sb, kT_sb, 1, 1, False)
        at(44, chunk_load, QT_d, qin, "q", 3)
        at(13, qk_proj, "k", wk_sb, kT_sb, 1, 2, False)
        at(14, qk_proj, "k", wk_sb, kT_sb, 1, 3, False)
        at(15, qk_proj, "k", wk_sb, kT_sb, 1, 4, False)
        at(18, qk_proj, "k", wk_sb, kT_sb, 1, 5, False)
        at(21, qk_proj, "k", wk_sb, kT_sb, 1, 6, False)
        at(24, qk_proj, "k", wk_sb, kT_sb, 1, 7, False)
        at(26, qk_proj, "q", wq_sb, qT_sb, 0, 2, True)
        at(28, qk_proj, "q", wq_sb, qT_sb, 0, 3, True)
        at(40, qk_proj, "q", wq_sb, qT_sb, 1, 2, True)
        at(42, qk_proj, "q", wq_sb, qT_sb, 1, 3, True)
        at(56, qk_proj, "q", wq_sb, qT_sb, 0, 4, True)
        at(58, qk_proj, "q", wq_sb, qT_sb, 0, 5, True)
        at(72, qk_proj, "q", wq_sb, qT_sb, 1, 4, True)
        at(74, qk_proj, "q", wq_sb, qT_sb, 1, 5, True)
        at(88, qk_proj, "q", wq_sb, qT_sb, 0, 6, True)
        at(90, qk_proj, "q", wq_sb, qT_sb, 0, 7, True)
        at(104, qk_proj, "q", wq_sb, qT_sb, 1, 6, True)
        at(106, qk_proj, "q", wq_sb, qT_sb, 1, 7, True)
        # out-proj spread through the back half to keep the PE backlog alive
        # (first use must follow the (qvb, hp1) sweep drain at ~32qvb+35)
        for qvb in range(3):
            for qs in range(4):
                at(48 + 26 * qvb + 6 * qs, out_proj, qvb, qs, False)

        j = 0  # ctx stream pointer into UNITS

        def ctx_lag(i):
            # sweep 0 trails 13..18 units (V still streaming; spreads the
            # vproj work), later sweeps 2; the 1.5-per-unit catch-up in the
            # emission loop decays the lag smoothly
            if i < NKC:
                return 13 + i // 2
            return 1 if i >= NU - NKC else 2

        # within a unit: injections and trailing ctx work go BEFORE the
        # scores pair — the in-order PE can chew on them while waiting for
        # exp(u-2) to free the scores psum tag. Mid-stream drain work (hh1
        # norm + transposes) defers 2-items-per-unit so it never wedges the
        # scores stream.
        def final_drain():
            # last sweep: ctx c15, batched norm, transposes, then dc-major
            # out-proj over the 6-slot tail rotation
            qvb, hp, c = UNITS[NU - 1]
            ctx_mm(qvb, hp, c)
            norm(qvb, hp)
            for qs in range(4):
                transpose(qvb, hp, qs, tail=True)
            o_sbs = [osb.tile([128, D], bf16, name="o_sb") for _ in range(4)]
            for dc in range(2):
                for qs in range(4):
                    qc = (512 * qvb + 128 * qs, 512 * qvb + 128 * (qs + 1))
                    o_ps = t_slot(512)
                    for cn in range(2):
                        nc.tensor.matmul(
                            o_ps[:], ctxT_sb[cn][:, qc[0]:qc[1]],
                            wo_sb[cn][:, 512 * dc:512 * (dc + 1)],
                            start=(cn == 0), stop=(cn == 1))
                    dst = o_sbs[qs][:, 512 * dc:512 * (dc + 1)]
                    if (2 * dc + qs) % 2 == 0:
                        nc.scalar.copy(dst, o_ps[:])
                    else:
                        nc.vector.tensor_copy(dst, o_ps[:])
                    eng = nc.sync if (dc + qs) % 2 == 0 else nc.gpsimd
                    eng.dma_start(
                        out_d[qc[0]:qc[1], 512 * dc:512 * (dc + 1)], dst)

        deferred = []
        for u in range(NU + 24):
            for fn, a in pre[u]:
                fn(*a)
            for _ in range(2):
                if deferred:
                    deferred.pop(0)()
            emitted = 0
            cap = 2 if u % 2 else 1
            while j < NU and j <= u - ctx_lag(j) and emitted < cap:
                qv2, hp2, c2 = UNITS[j]
                if j == NU - 1:
                    final_drain()
                    j += 1
                    break
                if j < NKC:
                    v_proj(c2)
                ctx_mm(qv2, hp2, c2)
                emitted += 1
                j += 1
                if c2 == NKC - 1:
                    norm_hh(qv2, hp2, 0)
                    deferred.append(
                        lambda q=qv2, h=hp2: norm_hh(q, h, 1))
                    for qs in range(4):
                        deferred.append(
                            lambda q=qv2, h=hp2, s=qs: transpose(q, h, s))
                    break  # don't cross a drain inside one unit
            if u < NU:
                scores_exp(*UNITS[u])
            if u >= NU and j >= NU:
                break

    nc.compile()
    return nc


def kernel(Q, K, V, wq, bq, wk, bk, wv, bv, wo, bo):
    import ml_dtypes
    from concourse.bass_utils import run_bass_kernel_spmd

    if "nc" not in _CACHE:
        _CACHE["nc"] = _build()
    nc = _CACHE["nc"]

    bf = ml_dtypes.bfloat16
    Q = np.asarray(Q, np.float32)
    K = np.asarray(K, np.float32)
    V = np.asarray(V, np.float32)
    QT = [np.ascontiguousarray(Q[b].T).astype(bf) for b in range(B)]
    KT = [np.ascontiguousarray(K[b].T).astype(bf) for b in range(B)]
    VT = [np.ascontiguousarray(V[b].T).astype(bf) for b in range(B)]
    def perm_qk(w, g):
        # [D, R] -> [128p, (hp, d, 128r)] with element [p,hp,d,r] =
        # wT[d*128+p, hp*128+r]
        wT = np.asarray(w, np.float32)[g * R:(g + 1) * R].T
        return np.ascontiguousarray(
            wT.reshape(8, 128, 2, 128).transpose(1, 2, 0, 3).reshape(128, 2048)
        ).astype(bf)

    def perm_v(w, g):
        wT = np.asarray(w, np.float32)[g * R:(g + 1) * R].T
        return np.ascontiguousarray(
            wT.reshape(8, 128, 256).transpose(1, 0, 2).reshape(128, 2048)
        ).astype(bf)

    wqP = [perm_qk(wq, g) for g in range(4)]
    wkP = [perm_qk(wk, g) for g in range(4)]
    wvP = [perm_v(wv, g) for g in range(4)]
    woT = [np.ascontiguousarray(np.asarray(wo, np.float32)[:, g * R:(g + 1) * R].T
                                ).astype(bf) for g in range(4)]
    bqs = [np.ascontiguousarray(np.asarray(bq, np.float32)[g * R:(g + 1) * R, None])
           for g in range(4)]
    ident = np.eye(128, dtype=np.float32)

    in_maps = []
    for c in range(NCORES):
        b, g = c // 4, c % 4
        in_maps.append({
            "QT": QT[b], "KT": KT[b], "VT": VT[b],
            "wqP": wqP[g], "wkP": wkP[g], "wvP": wvP[g], "woT": woT[g],
            "bq": bqs[g], "ident": ident,
        })

    global _LAST_IN_MAPS
    _LAST_IN_MAPS = in_maps
    res = run_bass_kernel_spmd(nc, in_maps, core_ids=list(range(NCORES)))

    host_bias = (np.asarray(bv, np.float32) @ np.asarray(wo, np.float32).T
                 + np.asarray(bo, np.float32))
    out = np.zeros((B, S, D), np.float32)
    for c in range(NCORES):
        out[c // 4] += np.asarray(res.results[c]["OUT"], np.float32)
    out += host_bias[None, None, :]
    return out


# revision 81
# speedup vs baseline: 1.0195x; 1.0074x over previous
"""MultiHeadAttention TRN2 kernel: B=2, S=2048, D=1024, H=16, DK=64, 8 cores.

Sharding: core c handles batch b=c//4 and heads hg=(c%4)*4 .. +3 (data + head
parallel). Projections are column-split by head; out-proj row-split; the
all-reduce after out-proj is done on host (sum of 4 partials per batch).

All activations/weights stream HBM<->SBUF as bf16 (host converts), halving
DMA on the serial DMA-engine resource. Matmul inputs are bf16 (1 cycle/row at
any moving size) except qT/kT which stay f32r for exp-input precision.

Device dataflow (per core):
  qT/kT = (w-slice).T @ QT/KT      -> [feat 128 (2 heads), seq] f32r, 256-wide
  v     = VT.T @ wv-slice          -> natural [kpos, 4*64] chunks -> v_all bf16
                                      [kpos, head*16*65] with ones col (den)
  scoresT[kpos,q] = kT-chunk.T @ qT  (K=64, both heads packed in one
                                      [128,1024] 2-bank PSUM tile)
  expT = exp(scoresT/8)            -> bf16 SBUF (ACT, the throughput floor)
  ctx[q, 65] += expT-slice.T @ v-chunk   (F=65 bf16, PSUM accum over kpos;
                                          col 64 accumulates the denominator)
  ctx_sb = ctx * recip(den)        -> [q, 128(2 heads)] f32 per qsub
  ctxT = PE-transpose(ctx_sb)      -> ctxT_sb [feat, q] bf16
  out[q, Dout] = ctxT.T @ wo       -> 256-wide chunks -> bf16 partial out

The whole thing is software-pipelined at DMA-chunk granularity: the hp0
weight halves + K s0 + Q s0 form a minimal first-exp prefix on the sync
queue (first exp at ~12us), then the remaining K/V/Q chunks stream in while
one global stream of 128 scores->exp units runs; ctx matmuls trail the exp
stream by 13..18 units during sweep 0 (V still arriving) then catch up to a
lag of 2; projections / out-proj / norms / transposes are injected between
units so the in-order PE never waits on data that hasn't arrived. The exp
stream runs back-to-back (1038 ns/tile) through the ACT-bound middle.

PSUM (8 banks): scA/scB [128,1024] x2 banks each (scores/exp dbuf) | ctxA,
ctxB [128,455] (7 of 8 per-sweep [128,65] accumulators, parity-alternating) |
ctxC [128,512] (8th accumulator per parity) | projC [128,512] (2 rotating
256-wide slots for q/k/v-proj, out-proj and ctx transposes).

Bias handling (exact): bq added on device (per-partition add in qT layout);
bk dropped (softmax shift-invariance); bv and bo folded on host as
out += bv @ wo.T + bo (softmax weights sum to 1).
"""

from contextlib import ExitStack

import numpy as np

B, S, D, H, DK = 2, 2048, 1024, 16, 64
NCORES = 8
HPC = H // (NCORES // B)      # heads per core = 4
R = HPC * DK                  # local feats = 256
NKC = S // 128                # 128-wide k chunks = 16
VW = 65                       # v chunk width (64 + ones col)

_CACHE = {}
_LAST_IN_MAPS = None


def _build():
    import concourse.mybir as mybir
    import concourse.tile as tile
    from concourse import bacc

    f32 = mybir.dt.float32
    f32r = mybir.dt.float32r
    bf16 = mybir.dt.bfloat16
    Exp = mybir.ActivationFunctionType.Exp
    Add = mybir.AluOpType.add
    Mult = mybir.AluOpType.mult

    nc = bacc.Bacc(
        "TRN2", target_bir_lowering=False, debug=False,
        enable_asserts=True, num_devices=NCORES,
    )

    QT_d = nc.dram_tensor("QT", [D, S], bf16, kind="ExternalInput").ap()
    KT_d = nc.dram_tensor("KT", [D, S], bf16, kind="ExternalInput").ap()
    VT_d = nc.dram_tensor("VT", [D, S], bf16, kind="ExternalInput").ap()
    # weights host-permuted to exact SBUF layout: wq/wk as [128, hp, d, 128]
    # (per-hp halves load separately, 2KB-contiguous rows), wv as [128, d, 256]
    wqP_d = nc.dram_tensor("wqP", [128, 2 * 8 * 128], bf16,
                           kind="ExternalInput").ap()
    wkP_d = nc.dram_tensor("wkP", [128, 2 * 8 * 128], bf16,
                           kind="ExternalInput").ap()
    wvP_d = nc.dram_tensor("wvP", [128, 8 * 256], bf16,
                           kind="ExternalInput").ap()
    woT_d = nc.dram_tensor("woT", [R, D], bf16, kind="ExternalInput").ap()
    bq_d = nc.dram_tensor("bq", [R, 1], f32, kind="ExternalInput").ap()
    id_d = nc.dram_tensor("ident", [128, 128], f32, kind="ExternalInput").ap()
    out_d = nc.dram_tensor("OUT", [S, D], bf16, kind="ExternalOutput").ap()

    with tile.TileContext(nc) as tc, ExitStack() as ctx:
        sb = ctx.enter_context(tc.tile_pool(name="sb", bufs=1))
        qin = ctx.enter_context(tc.tile_pool(name="qin", bufs=6))
        kin = ctx.enter_context(tc.tile_pool(name="kin", bufs=8))
        vin = ctx.enter_context(tc.tile_pool(name="vin", bufs=6))
        expp = ctx.enter_context(tc.tile_pool(name="expp", bufs=26))
        cxp = ctx.enter_context(tc.tile_pool(name="cxp", bufs=2))
        osb = ctx.enter_context(tc.tile_pool(name="osb", bufs=4))
        psum = ctx.enter_context(tc.tile_pool(name="psum", bufs=1, space="PSUM"))

        # ---- persistent PSUM containers (8 banks exactly) ----
        # PSUM accumulation groups are zero-region (= bank) granular: a
        # start_tensor_calc matmul zeroes its whole bank, so each bank holds
        # exactly one live group. Scores halves are full banks; the 4 ctx
        # accumulators of one hh live in one bank as a single group; proj /
        # out-proj / transpose rotate through two whole-bank slots.
        sc_ps = [psum.tile([128, 1024], f32, name=f"sc{i}") for i in range(2)]
        ctxH = [psum.tile([128, 260], f32, name=f"ctxh{i}") for i in range(2)]
        projAB = [psum.tile([128, 512], f32, name=f"proj{i}") for i in range(2)]

        cnt = {"p": 0}

        def p_slot(w):
            # rotating whole-bank psum slot for q/k/v-proj, out-proj and
            # transposes; overlapping-view hazards serialize reuse
            i = cnt["p"]; cnt["p"] += 1
            return projAB[i % 2][:, 0:w]

        # ---- persistent SBUF ----
        wq_sb = sb.tile([128, 8 * R], bf16)   # (hp, d) block at 1024*hp+128*d
        wk_sb = sb.tile([128, 8 * R], bf16)
        wv_sb = sb.tile([128, 8 * R], bf16)   # D-chunk d at cols [R*d : +R]
        wo_sb = [sb.tile([128, D], bf16, name=f"wo_sb{cn}") for cn in range(2)]
        bq_sb = sb.tile([128, 2], f32)
        id_sb = sb.tile([128, 128], f32)

        qT_sb = [sb.tile([128, S], bf16, name=f"qT_sb{hp}") for hp in range(2)]
        kT_sb = [sb.tile([128, S], bf16, name=f"kT_sb{hp}") for hp in range(2)]
        v_all = sb.tile([128, HPC * NKC * VW], bf16)  # (h, c) at (h*NKC+c)*VW
        ctxT_sb = [sb.tile([128, S], bf16, name=f"ctxT_sb{cn}") for cn in range(2)]

        onecol = sb.tile([128, 1], f32)
        nc.vector.memset(onecol[:], 1.0)
        vv = v_all.rearrange("p (n c) -> p n c", c=VW)[:, :, 64:65].rearrange(
            "p n c -> p (n c)")
        nc.vector.tensor_copy(vv, onecol[:].broadcast_to((128, HPC * NKC)))

        def w_load(w_sb, w_d, hp=None):
            if hp is None:
                nc.sync.dma_start(w_sb[:], w_d[:, :])
            else:
                nc.sync.dma_start(w_sb[:, 1024 * hp:1024 * (hp + 1)],
                                  w_d[:, 1024 * hp:1024 * (hp + 1)])

        # staging tiles: one [128, 2048] bf16 tile covers 4 d-chunks x 512
        # seq; a (tensor, sblk) pair = 2 tiles (d 0-3, d 4-7)
        stage = {}

        def chunk_load(src, pool, tag, sblk):
            tiles = []
            for hf in range(2):
                t = pool.tile([128, 2048], bf16, name=tag, tag=tag)
                nc.sync.dma_start(
                    t.rearrange("p (d s) -> p d s", d=4),
                    src.rearrange("(d p) s -> p d s", p=128)[
                        :, 4 * hf:4 * hf + 4, 512 * sblk:512 * (sblk + 1)])
                tiles.append(t)
            stage[(tag, sblk)] = ("std", tiles)

        def first_load(src, pool, tag):
            # s0 of K/Q as two all-d x 256-col chunks so the first proj
            # tile only waits for half the data
            tiles = []
            for half in range(2):
                t = pool.tile([128, 2048], bf16, name=tag, tag=tag)
                nc.sync.dma_start(
                    t.rearrange("p (d s) -> p d s", d=8),
                    src.rearrange("(d p) s -> p d s", p=128)[
                        :, :, 256 * half:256 * (half + 1)])
                tiles.append(t)
            stage[(tag, 0)] = ("first", tiles)

        def staged(tag, sblk, d, cols):
            # d-chunk d of sblk, column slice `cols` within the 512-wide sblk
            ent = stage[(tag, sblk)]
            if ent[0] == "first":
                half = cols[0] // 256
                off = cols[0] - 256 * half
                return ent[1][half][
                    :, 256 * d + off:256 * d + off + cols[1] - cols[0]]
            t = ent[1][d // 4]
            base = 512 * (d % 4)
            return t[:, base + cols[0]:base + cols[1]]

        # ---- projection tiles ----
        def qk_proj(tag, w_sb, dst_sb, hp, j, bias):
            # (hp, j): 256 seq cols [256j : 256j+256] of head-pair hp
            sblk, half = j // 2, j % 2
            cols = (256 * half, 256 * half + 256)
            p_ps = p_slot(256)
            for d in range(8):
                nc.tensor.matmul(
                    p_ps[:],
                    w_sb[:, 1024 * hp + 128 * d:1024 * hp + 128 * (d + 1)],
                    staged(tag, sblk, d, cols), start=(d == 0), stop=(d == 7))
            dst = dst_sb[hp][:, 256 * j:256 * (j + 1)]
            if bias:
                nc.vector.tensor_scalar(
                    dst, p_ps[:], bq_sb[:, hp:hp + 1], None, op0=Add)
            else:
                nc.vector.tensor_copy(dst, p_ps[:])

        def v_proj(c):
            # kpos chunk c (128 rows): out [kpos, 256 feats] -> v_all slices
            sblk, sub = c // 4, c % 4
            cols = (128 * sub, 128 * sub + 128)
            v_ps = p_slot(256)
            for d in range(8):
                nc.tensor.matmul(
                    v_ps[:], staged("v", sblk, d, cols),
                    wv_sb[:, R * d:R * (d + 1)], start=(d == 0), stop=(d == 7))
            va = v_all.rearrange("p (h n c) -> p h n c", h=HPC, n=NKC)
            nc.vector.tensor_copy(
                va[:, :, c:c + 1, 0:64],
                v_ps[:].rearrange("p (h n c) -> p h n c", h=HPC, n=1))

        # ---- attention sweep pieces (scores/exp stream + trailing ctx) ----
        exp_ring = {}

        def scores_exp(qvb, hp, c):
            s_ps = sc_ps[c % 2]
            for hh in range(2):
                nc.tensor.matmul(
                    s_ps[:, 512 * hh:512 * (hh + 1)],
                    kT_sb[hp][64 * hh:64 * (hh + 1), 128 * c:128 * (c + 1)],
                    qT_sb[hp][64 * hh:64 * (hh + 1), 512 * qvb:512 * (qvb + 1)],
                    start=True, stop=True)
            expT = expp.tile([128, 1024], bf16, name="expT")
            nc.scalar.activation(expT[:], s_ps[:], Exp, scale=0.125)
            exp_ring[(qvb, hp, c)] = expT

        def ctx_mm(qvb, hp, c):
            expT = exp_ring.pop((qvb, hp, c))
            for hh in range(2):
                gh = HPC // 2 * hp + hh
                for qs in range(4):
                    nc.tensor.matmul(
                        ctxH[hh][0:128, 65 * qs:65 * qs + VW],
                        expT[:, 512 * hh + 128 * qs:512 * hh + 128 * (qs + 1)],
                        v_all[:, (gh * NKC + c) * VW:(gh * NKC + c + 1) * VW],
                        start=(c == 0 and qs == 0),
                        stop=(c == NKC - 1 and qs == 3))

        ctx_stage = {}

        def norm_hh(qvb, hp, hh):
            # drain one ctxH bank: a strided recip over the 4 denominator
            # columns + one strided multiply into the (qs, hh, 64) staging
            # tile cs
            if hh == 0:
                ctx_stage[(qvb, hp)] = cxp.tile(
                    [128, 512], f32, name="ctxs", tag=f"ctxs{hp}")
            cs = ctx_stage[(qvb, hp)]
            t3 = ctxH[hh].rearrange("p (qs w) -> p qs w", w=VW)
            rb = cxp.tile([128, 4], f32, name="rb", tag=f"rb{hh}")
            nc.vector.reciprocal_approx_fast(
                out=rb[:], in_=t3[:, :, 64:65].rearrange("p a b -> p (a b)"))
            nc.vector.tensor_mul(
                cs.rearrange("p (qs hh f) -> p qs hh f", qs=4, hh=2)[
                    :, :, hh, :],
                t3[:, :, 0:64],
                rb.rearrange("p (a b) -> p a b", b=1).broadcast_to(
                    (128, 4, 64)))

        def norm(qvb, hp):
            norm_hh(qvb, hp, 0)
            norm_hh(qvb, hp, 1)

        def transpose(qvb, hp, qs, tail=False):
            cs = ctx_stage[(qvb, hp)]
            lhsT = cs[:, 128 * qs:128 * (qs + 1)]
            tp = t_slot(128) if tail else p_slot(128)
            nc.tensor.matmul(tp, lhsT, id_sb[:], is_transpose=True)
            # after the last exp ACT is idle; split evacs across ACT and DVE
            dst = ctxT_sb[hp][:, 512 * qvb + 128 * qs:512 * qvb + 128 * (qs + 1)]
            if tail and qs % 2 == 0:
                nc.scalar.copy(dst, tp)
            else:
                nc.vector.tensor_copy(dst, tp)
            if qs == 3:
                del ctx_stage[(qvb, hp)]

        # after the final exp the score banks are free: the tail out-proj /
        # transposes rotate over 4 whole-bank psum slots instead of 2
        tail_slots = [lambda w: projAB[0][:, 0:w], lambda w: projAB[1][:, 0:w],
                      lambda w: sc_ps[0][:, 0:w], lambda w: sc_ps[0][:, 512:512 + w],
                      lambda w: sc_ps[1][:, 0:w], lambda w: sc_ps[1][:, 512:512 + w]]

        def t_slot(w):
            i = cnt["p"]; cnt["p"] += 1
            return tail_slots[i % 6](w)

        def out_proj(qvb, qs, tail=False):
            o_sb = osb.tile([128, D], bf16, name="o_sb")
            qcols = (512 * qvb + 128 * qs, 512 * qvb + 128 * (qs + 1))
            for dc in range(2):
                o_ps = t_slot(512) if tail else p_slot(512)
                for cn in range(2):
                    nc.tensor.matmul(
                        o_ps[:], ctxT_sb[cn][:, qcols[0]:qcols[1]],
                        wo_sb[cn][:, 512 * dc:512 * (dc + 1)],
                        start=(cn == 0), stop=(cn == 1))
                if tail and dc % 2 == 0:
                    nc.scalar.copy(o_sb[:, 512 * dc:512 * (dc + 1)], o_ps[:])
                else:
                    nc.vector.tensor_copy(o_sb[:, 512 * dc:512 * (dc + 1)],
                                          o_ps[:])
                # half-store right after its evac, alternating queues so the
                # final stores drain two DGE pipelines in parallel; the last
                # store goes on sync (no gpsimd Q7 launch on the tail)
                eng = nc.gpsimd if dc == 0 else nc.sync
                eng.dma_start(
                    out_d[qcols[0]:qcols[1], 512 * dc:512 * (dc + 1)],
                    o_sb[:, 512 * dc:512 * (dc + 1)])

        # ================= emission schedule =================
        # One global stream of 128 exp units ((qvb, hp) sweeps, kchunk
        # minor). ctx matmuls trail by 13 units during sweep 0 (V still
        # streaming), then catch up 2-per-unit to a lag of 2 so the tail
        # stays short.
        UNITS = [(s // 2, s % 2, c) for s in range(8) for c in range(NKC)]
        NU = len(UNITS)
        pre = {u: [] for u in range(NU + 16)}

        def at(u, fn, *a):
            pre[u].append((fn, a))

        def wo_load():
            for cn in range(2):
                nc.sync.dma_start(wo_sb[cn][:], woT_d[128 * cn:128 * (cn + 1), :])

        # PE warmup: keep the tensor engine busy (and its p-state ramp hot)
        # through the DMA-bound prologue; calibrated to end near Ks0 arrival
        wsc = sb.tile([128, 512], bf16)
        nc.vector.memset(wsc[:], 1.0)

        def warmup(n):
            # rotate over all 6 tail slots so the WAW chain never paces the
            # warmup below the engine rate
            for _ in range(n):
                wp = t_slot(512)
                nc.tensor.matmul(wp, wsc[:, 0:128], wsc[:], start=True, stop=True)

        # prologue DMAs (sync queue order = arrival order): hp0 weight
        # halves + K s0 + Q s0 form the minimal first-exp prefix
        w_load(wk_sb, wkP_d, 0)
        w_load(wq_sb, wqP_d, 0)
        first_load(KT_d, kin, "k")
        first_load(QT_d, qin, "q")
        nc.sync.dma_start(
            bq_sb.rearrange("p (hp c) -> p hp c", hp=2),
            bq_d.rearrange("(hp p) c -> p hp c", p=128))
        chunk_load(KT_d, kin, "k", 1)
        nc.sync.dma_start(id_sb[:], id_d[:, :])
        w_load(wv_sb, wvP_d)
        w_load(wk_sb, wkP_d, 1)
        w_load(wq_sb, wqP_d, 1)
        qk_proj("k", wk_sb, kT_sb, 0, 0, False)
        qk_proj("k", wk_sb, kT_sb, 0, 1, False)
        qk_proj("q", wq_sb, qT_sb, 0, 0, True)
        qk_proj("q", wq_sb, qT_sb, 0, 1, True)

        at(2, chunk_load, KT_d, kin, "k", 2)
        at(2, qk_proj, "k", wk_sb, kT_sb, 0, 2, False)
        at(3, chunk_load, KT_d, kin, "k", 3)
        at(3, qk_proj, "k", wk_sb, kT_sb, 0, 3, False)
        at(4, chunk_load, VT_d, vin, "v", 0)
        at(5, chunk_load, VT_d, vin, "v", 1)
        at(5, qk_proj, "k", wk_sb, kT_sb, 0, 4, False)
        at(6, qk_proj, "k", wk_sb, kT_sb, 0, 5, False)
        at(6, chunk_load, VT_d, vin, "v", 2)
        at(7, qk_proj, "q", wq_sb, qT_sb, 1, 0, True)
        at(7, chunk_load, VT_d, vin, "v", 3)
        at(8, qk_proj, "k", wk_sb, kT_sb, 0, 6, False)
        at(9, qk_proj, "k", wk_sb, kT_sb, 0, 7, False)
        at(9, chunk_load, QT_d, qin, "q", 1)
        at(10, qk_proj, "q", wq_sb, qT_sb, 1, 1, True)
        at(10, wo_load)
        at(11, chunk_load, QT_d, qin, "q", 2)
        at(11, qk_proj, "k", wk_sb, kT_sb, 1, 0, False)
        at(12, qk_proj, "k", wk_sb, kT_sb, 1, 1, False)
        at(44, chunk_load, QT_d, qin, "q", 3)
        at(13, qk_proj, "k", wk_sb, kT_sb, 1, 2, False)
        at(14, qk_proj, "k", wk_sb, kT_sb, 1, 3, False)
        at(15, qk_proj, "k", wk_sb, kT_sb, 1, 4, False)
        at(18, qk_proj, "k", wk_sb, kT_sb, 1, 5, False)
        at(21, qk_proj, "k", wk_sb, kT_sb, 1, 6, False)
        at(24, qk_proj, "k", wk_sb, kT_sb, 1, 7, False)
        at(26, qk_proj, "q", wq_sb, qT_sb, 0, 2, True)
        at(28, qk_proj, "q", wq_sb, qT_sb, 0, 3, True)
        at(40, qk_proj, "q", wq_sb, qT_sb, 1, 2, True)
        at(42, qk_proj, "q", wq_sb, qT_sb, 1, 3, True)
        at(56, qk_proj, "q", wq_sb, qT_sb, 0, 4, True)
        at(58, qk_proj, "q", wq_sb, qT_sb, 0, 5, True)
        at(72, qk_proj, "q", wq_sb, qT_sb, 1, 4, True)
        at(74, qk_proj, "q", wq_sb, qT_sb, 1, 5, True)
        at(88, qk_proj, "q", wq_sb, qT_sb, 0, 6, True)
        at(90, qk_proj, "q", wq_sb, qT_sb, 0, 7, True)
        at(104, qk_proj, "q", wq_sb, qT_sb, 1, 6, True)
        at(106, qk_proj, "q", wq_sb, qT_sb, 1, 7, True)
        # out-proj spread through the back half to keep the PE backlog alive
        # (first use must follow the (qvb, hp1) sweep drain at ~32qvb+35)
        for qvb in range(3):
            for qs in range(4):
                at(52 + 26 * qvb + 6 * qs, out_proj, qvb, qs, False)

        j = 0  # ctx stream pointer into UNITS

        def ctx_lag(i):
            # sweep 0 trails 13..18 units (V still streaming; spreads the
            # vproj work), later sweeps 2; the 1.5-per-unit catch-up in the
            # emission loop decays the lag smoothly
            if i < NKC:
                return 13 + 2 * i // 3
            return 1 if i >= NU - NKC else 2

        # within a unit: injections and trailing ctx work go BEFORE the
        # scores pair — the in-order PE can chew on them while waiting for
        # exp(u-2) to free the scores psum tag. Mid-stream drain work (hh1
        # norm + transposes) defers 2-items-per-unit so it never wedges the
        # scores stream.
        def final_drain():
            # last sweep: ctx c15, batched norm, transposes, then dc-major
            # out-proj over the 6-slot tail rotation
            qvb, hp, c = UNITS[NU - 1]
            ctx_mm(qvb, hp, c)
            norm(qvb, hp)
            for qs in range(4):
                transpose(qvb, hp, qs, tail=True)
            o_sbs = [osb.tile([128, D], bf16, name="o_sb") for _ in range(4)]
            for dc in range(2):
                for qs in range(4):
                    qc = (512 * qvb + 128 * qs, 512 * qvb + 128 * (qs + 1))
                    o_ps = t_slot(512)
                    for cn in range(2):
                        nc.tensor.matmul(
                            o_ps[:], ctxT_sb[cn][:, qc[0]:qc[1]],
                            wo_sb[cn][:, 512 * dc:512 * (dc + 1)],
                            start=(cn == 0), stop=(cn == 1))
                    dst = o_sbs[qs][:, 512 * dc:512 * (dc + 1)]
                    if (2 * dc + qs) % 2 == 0:
                        nc.scalar.copy(dst, o_ps[:])
                    else:
                        nc.vector.tensor_copy(dst, o_ps[:])
                    eng = nc.sync if (dc + qs) % 2 == 0 else nc.gpsimd
                    eng.dma_start(
                        out_d[qc[0]:qc[1], 512 * dc:512 * (dc + 1)], dst)

        deferred = []
        for u in range(NU + 24):
            for fn, a in pre[u]:
                fn(*a)
            for _ in range(2):
                if deferred:
                    deferred.pop(0)()
            emitted = 0
            cap = 2 if u % 2 else 1
            while j < NU and j <= u - ctx_lag(j) and emitted < cap:
                qv2, hp2, c2 = UNITS[j]
                if j == NU - 1:
                    final_drain()
                    j += 1
                    break
                if j < NKC:
                    v_proj(c2)
                ctx_mm(qv2, hp2, c2)
                emitted += 1
                j += 1
                if c2 == NKC - 1:
                    norm_hh(qv2, hp2, 0)
                    deferred.append(
                        lambda q=qv2, h=hp2: norm_hh(q, h, 1))
                    for qs in range(4):
                        deferred.append(
                            lambda q=qv2, h=hp2, s=qs: transpose(q, h, s))
                    break  # don't cross a drain inside one unit
            if u < NU:
                scores_exp(*UNITS[u])
            if u >= NU and j >= NU:
                break

    nc.compile()
    return nc


def kernel(Q, K, V, wq, bq, wk, bk, wv, bv, wo, bo):
    import ml_dtypes
    from concourse.bass_utils import run_bass_kernel_spmd

    if "nc" not in _CACHE:
        _CACHE["nc"] = _build()
    nc = _CACHE["nc"]

    bf = ml_dtypes.bfloat16
    Q = np.asarray(Q, np.float32)
    K = np.asarray(K, np.float32)
    V = np.asarray(V, np.float32)
    QT = [np.ascontiguousarray(Q[b].T).astype(bf) for b in range(B)]
    KT = [np.ascontiguousarray(K[b].T).astype(bf) for b in range(B)]
    VT = [np.ascontiguousarray(V[b].T).astype(bf) for b in range(B)]
    def perm_qk(w, g):
        # [D, R] -> [128p, (hp, d, 128r)] with element [p,hp,d,r] =
        # wT[d*128+p, hp*128+r]
        wT = np.asarray(w, np.float32)[g * R:(g + 1) * R].T
        return np.ascontiguousarray(
            wT.reshape(8, 128, 2, 128).transpose(1, 2, 0, 3).reshape(128, 2048)
        ).astype(bf)

    def perm_v(w, g):
        wT = np.asarray(w, np.float32)[g * R:(g + 1) * R].T
        return np.ascontiguousarray(
            wT.reshape(8, 128, 256).transpose(1, 0, 2).reshape(128, 2048)
        ).astype(bf)

    wqP = [perm_qk(wq, g) for g in range(4)]
    wkP = [perm_qk(wk, g) for g in range(4)]
    wvP = [perm_v(wv, g) for g in range(4)]
    woT = [np.ascontiguousarray(np.asarray(wo, np.float32)[:, g * R:(g + 1) * R].T
                                ).astype(bf) for g in range(4)]
    bqs = [np.ascontiguousarray(np.asarray(bq, np.float32)[g * R:(g + 1) * R, None])
           for g in range(4)]
    ident = np.eye(128, dtype=np.float32)

    in_maps = []
    for c in range(NCORES):
        b, g = c // 4, c % 4
        in_maps.append({
            "QT": QT[b], "KT": KT[b], "VT": VT[b],
            "wqP": wqP[g], "wkP": wkP[g], "wvP": wvP[g], "woT": woT[g],
            "bq": bqs[g], "ident": ident,
        })

    global _LAST_IN_MAPS
    _LAST_IN_MAPS = in_maps
    res = run_bass_kernel_spmd(nc, in_maps, core_ids=list(range(NCORES)))

    host_bias = (np.asarray(bv, np.float32) @ np.asarray(wo, np.float32).T
                 + np.asarray(bo, np.float32))
    out = np.zeros((B, S, D), np.float32)
    for c in range(NCORES):
        out[c // 4] += np.asarray(res.results[c]["OUT"], np.float32)
    out += host_bias[None, None, :]
    return out


# revision 82
# speedup vs baseline: 1.0396x; 1.0197x over previous
"""MultiHeadAttention TRN2 kernel: B=2, S=2048, D=1024, H=16, DK=64, 8 cores.

Sharding: core c handles batch b=c//4 and heads hg=(c%4)*4 .. +3 (data + head
parallel). Projections are column-split by head; out-proj row-split; the
all-reduce after out-proj is done on host (sum of 4 partials per batch).

All activations/weights stream HBM<->SBUF as bf16 (host converts), halving
DMA on the serial DMA-engine resource. Matmul inputs are bf16 (1 cycle/row at
any moving size) except qT/kT which stay f32r for exp-input precision.

Device dataflow (per core):
  qT/kT = (w-slice).T @ QT/KT      -> [feat 128 (2 heads), seq] f32r, 256-wide
  v     = VT.T @ wv-slice          -> natural [kpos, 4*64] chunks -> v_all bf16
                                      [kpos, head*16*65] with ones col (den)
  scoresT[kpos,q] = kT-chunk.T @ qT  (K=64, both heads packed in one
                                      [128,1024] 2-bank PSUM tile)
  expT = exp(scoresT/8)            -> bf16 SBUF (ACT, the throughput floor)
  ctx[q, 65] += expT-slice.T @ v-chunk   (F=65 bf16, PSUM accum over kpos;
                                          col 64 accumulates the denominator)
  ctx_sb = ctx * recip(den)        -> [q, 128(2 heads)] f32 per qsub
  ctxT = PE-transpose(ctx_sb)      -> ctxT_sb [feat, q] bf16
  out[q, Dout] = ctxT.T @ wo       -> 256-wide chunks -> bf16 partial out

The whole thing is software-pipelined at DMA-chunk granularity: the hp0
weight halves + K s0 + Q s0 form a minimal first-exp prefix on the sync
queue (first exp at ~12us), then the remaining K/V/Q chunks stream in while
one global stream of 128 scores->exp units runs; ctx matmuls trail the exp
stream by 13..18 units during sweep 0 (V still arriving) then catch up to a
lag of 2; projections / out-proj / norms / transposes are injected between
units so the in-order PE never waits on data that hasn't arrived. The exp
stream runs back-to-back (1038 ns/tile) through the ACT-bound middle.

PSUM (8 banks): scA/scB [128,1024] x2 banks each (scores/exp dbuf) | ctxA,
ctxB [128,455] (7 of 8 per-sweep [128,65] accumulators, parity-alternating) |
ctxC [128,512] (8th accumulator per parity) | projC [128,512] (2 rotating
256-wide slots for q/k/v-proj, out-proj and ctx transposes).

Bias handling (exact): bq added on device (per-partition add in qT layout);
bk dropped (softmax shift-invariance); bv and bo folded on host as
out += bv @ wo.T + bo (softmax weights sum to 1).
"""

from contextlib import ExitStack

import numpy as np

B, S, D, H, DK = 2, 2048, 1024, 16, 64
NCORES = 8
HPC = H // (NCORES // B)      # heads per core = 4
R = HPC * DK                  # local feats = 256
NKC = S // 128                # 128-wide k chunks = 16
VW = 65                       # v chunk width (64 + ones col)

_CACHE = {}
_LAST_IN_MAPS = None


def _build():
    import concourse.mybir as mybir
    import concourse.tile as tile
    from concourse import bacc

    f32 = mybir.dt.float32
    f32r = mybir.dt.float32r
    bf16 = mybir.dt.bfloat16
    Exp = mybir.ActivationFunctionType.Exp
    Add = mybir.AluOpType.add
    Mult = mybir.AluOpType.mult

    nc = bacc.Bacc(
        "TRN2", target_bir_lowering=False, debug=False,
        enable_asserts=True, num_devices=NCORES,
    )

    QT_d = nc.dram_tensor("QT", [D, S], bf16, kind="ExternalInput").ap()
    KT_d = nc.dram_tensor("KT", [D, S], bf16, kind="ExternalInput").ap()
    VT_d = nc.dram_tensor("VT", [D, S], bf16, kind="ExternalInput").ap()
    # weights host-permuted to exact SBUF layout: wq/wk as [128, hp, d, 128]
    # (per-hp halves load separately, 2KB-contiguous rows), wv as [128, d, 256]
    wqP_d = nc.dram_tensor("wqP", [128, 2 * 8 * 128], bf16,
                           kind="ExternalInput").ap()
    wkP_d = nc.dram_tensor("wkP", [128, 2 * 8 * 128], bf16,
                           kind="ExternalInput").ap()
    wvP_d = nc.dram_tensor("wvP", [128, 8 * 256], bf16,
                           kind="ExternalInput").ap()
    woT_d = nc.dram_tensor("woT", [R, D], bf16, kind="ExternalInput").ap()
    bq_d = nc.dram_tensor("bq", [R, 1], f32, kind="ExternalInput").ap()
    id_d = nc.dram_tensor("ident", [128, 128], f32, kind="ExternalInput").ap()
    out_d = nc.dram_tensor("OUT", [S, D], bf16, kind="ExternalOutput").ap()

    with tile.TileContext(nc) as tc, ExitStack() as ctx:
        sb = ctx.enter_context(tc.tile_pool(name="sb", bufs=1))
        qin = ctx.enter_context(tc.tile_pool(name="qin", bufs=6))
        kin = ctx.enter_context(tc.tile_pool(name="kin", bufs=8))
        vin = ctx.enter_context(tc.tile_pool(name="vin", bufs=6))
        expp = ctx.enter_context(tc.tile_pool(name="expp", bufs=31))
        cxp = ctx.enter_context(tc.tile_pool(name="cxp", bufs=2))
        osb = ctx.enter_context(tc.tile_pool(name="osb", bufs=4))
        psum = ctx.enter_context(tc.tile_pool(name="psum", bufs=1, space="PSUM"))

        # ---- persistent PSUM containers (8 banks exactly) ----
        # PSUM accumulation groups are zero-region (= bank) granular: a
        # start_tensor_calc matmul zeroes its whole bank, so each bank holds
        # exactly one live group. Scores halves are full banks; the 4 ctx
        # accumulators of one hh live in one bank as a single group; proj /
        # out-proj / transpose rotate through two whole-bank slots.
        sc_ps = [psum.tile([128, 1024], f32, name=f"sc{i}") for i in range(2)]
        ctxH = [psum.tile([128, 260], f32, name=f"ctxh{i}") for i in range(2)]
        projAB = [psum.tile([128, 512], f32, name=f"proj{i}") for i in range(2)]

        cnt = {"p": 0}

        def p_slot(w):
            # rotating whole-bank psum slot for q/k/v-proj, out-proj and
            # transposes; overlapping-view hazards serialize reuse
            i = cnt["p"]; cnt["p"] += 1
            return projAB[i % 2][:, 0:w]

        # ---- persistent SBUF ----
        wq_sb = sb.tile([128, 8 * R], bf16)   # (hp, d) block at 1024*hp+128*d
        wk_sb = sb.tile([128, 8 * R], bf16)
        wv_sb = sb.tile([128, 8 * R], bf16)   # D-chunk d at cols [R*d : +R]
        wo_sb = [sb.tile([128, D], bf16, name=f"wo_sb{cn}") for cn in range(2)]
        bq_sb = sb.tile([128, 2], f32)
        id_sb = sb.tile([128, 128], f32)

        qT_sb = [sb.tile([128, S], bf16, name=f"qT_sb{hp}") for hp in range(2)]
        kT_sb = [sb.tile([128, S], bf16, name=f"kT_sb{hp}") for hp in range(2)]
        v_all = sb.tile([128, HPC * NKC * VW], bf16)  # (h, c) at (h*NKC+c)*VW
        ctxT_sb = [sb.tile([128, S], bf16, name=f"ctxT_sb{cn}") for cn in range(2)]

        onecol = sb.tile([128, 1], f32)
        nc.vector.memset(onecol[:], 1.0)
        vv = v_all.rearrange("p (n c) -> p n c", c=VW)[:, :, 64:65].rearrange(
            "p n c -> p (n c)")
        nc.vector.tensor_copy(vv, onecol[:].broadcast_to((128, HPC * NKC)))

        def w_load(w_sb, w_d, hp=None):
            if hp is None:
                nc.sync.dma_start(w_sb[:], w_d[:, :])
            else:
                nc.sync.dma_start(w_sb[:, 1024 * hp:1024 * (hp + 1)],
                                  w_d[:, 1024 * hp:1024 * (hp + 1)])

        # staging tiles: one [128, 2048] bf16 tile covers 4 d-chunks x 512
        # seq; a (tensor, sblk) pair = 2 tiles (d 0-3, d 4-7)
        stage = {}

        def chunk_load(src, pool, tag, sblk):
            tiles = []
            for hf in range(2):
                t = pool.tile([128, 2048], bf16, name=tag, tag=tag)
                nc.sync.dma_start(
                    t.rearrange("p (d s) -> p d s", d=4),
                    src.rearrange("(d p) s -> p d s", p=128)[
                        :, 4 * hf:4 * hf + 4, 512 * sblk:512 * (sblk + 1)])
                tiles.append(t)
            stage[(tag, sblk)] = ("std", tiles)

        def first_load(src, pool, tag):
            # s0 of K/Q as two all-d x 256-col chunks so the first proj
            # tile only waits for half the data
            tiles = []
            for half in range(2):
                t = pool.tile([128, 2048], bf16, name=tag, tag=tag)
                nc.sync.dma_start(
                    t.rearrange("p (d s) -> p d s", d=8),
                    src.rearrange("(d p) s -> p d s", p=128)[
                        :, :, 256 * half:256 * (half + 1)])
                tiles.append(t)
            stage[(tag, 0)] = ("first", tiles)

        def staged(tag, sblk, d, cols):
            # d-chunk d of sblk, column slice `cols` within the 512-wide sblk
            ent = stage[(tag, sblk)]
            if ent[0] == "first":
                half = cols[0] // 256
                off = cols[0] - 256 * half
                return ent[1][half][
                    :, 256 * d + off:256 * d + off + cols[1] - cols[0]]
            t = ent[1][d // 4]
            base = 512 * (d % 4)
            return t[:, base + cols[0]:base + cols[1]]

        # ---- projection tiles ----
        def qk_proj(tag, w_sb, dst_sb, hp, j, bias):
            # (hp, j): 256 seq cols [256j : 256j+256] of head-pair hp
            sblk, half = j // 2, j % 2
            cols = (256 * half, 256 * half + 256)
            p_ps = p_slot(256)
            for d in range(8):
                nc.tensor.matmul(
                    p_ps[:],
                    w_sb[:, 1024 * hp + 128 * d:1024 * hp + 128 * (d + 1)],
                    staged(tag, sblk, d, cols), start=(d == 0), stop=(d == 7))
            dst = dst_sb[hp][:, 256 * j:256 * (j + 1)]
            if bias:
                nc.vector.tensor_scalar(
                    dst, p_ps[:], bq_sb[:, hp:hp + 1], None, op0=Add)
            else:
                nc.vector.tensor_copy(dst, p_ps[:])

        def v_proj(c):
            # kpos chunk c (128 rows): out [kpos, 256 feats] -> v_all slices
            sblk, sub = c // 4, c % 4
            cols = (128 * sub, 128 * sub + 128)
            v_ps = p_slot(256)
            for d in range(8):
                nc.tensor.matmul(
                    v_ps[:], staged("v", sblk, d, cols),
                    wv_sb[:, R * d:R * (d + 1)], start=(d == 0), stop=(d == 7))
            va = v_all.rearrange("p (h n c) -> p h n c", h=HPC, n=NKC)
            nc.vector.tensor_copy(
                va[:, :, c:c + 1, 0:64],
                v_ps[:].rearrange("p (h n c) -> p h n c", h=HPC, n=1))

        # ---- attention sweep pieces (scores/exp stream + trailing ctx) ----
        exp_ring = {}

        def scores_exp(qvb, hp, c):
            s_ps = sc_ps[c % 2]
            for hh in range(2):
                nc.tensor.matmul(
                    s_ps[:, 512 * hh:512 * (hh + 1)],
                    kT_sb[hp][64 * hh:64 * (hh + 1), 128 * c:128 * (c + 1)],
                    qT_sb[hp][64 * hh:64 * (hh + 1), 512 * qvb:512 * (qvb + 1)],
                    start=True, stop=True)
            expT = expp.tile([128, 1024], bf16, name="expT")
            nc.scalar.activation(expT[:], s_ps[:], Exp, scale=0.125)
            exp_ring[(qvb, hp, c)] = expT

        def ctx_mm(qvb, hp, c):
            expT = exp_ring.pop((qvb, hp, c))
            for hh in range(2):
                gh = HPC // 2 * hp + hh
                for qs in range(4):
                    nc.tensor.matmul(
                        ctxH[hh][0:128, 65 * qs:65 * qs + VW],
                        expT[:, 512 * hh + 128 * qs:512 * hh + 128 * (qs + 1)],
                        v_all[:, (gh * NKC + c) * VW:(gh * NKC + c + 1) * VW],
                        start=(c == 0 and qs == 0),
                        stop=(c == NKC - 1 and qs == 3))

        ctx_stage = {}

        def norm_hh(qvb, hp, hh):
            # drain one ctxH bank: a strided recip over the 4 denominator
            # columns + one strided multiply into the (qs, hh, 64) staging
            # tile cs
            if hh == 0:
                ctx_stage[(qvb, hp)] = cxp.tile(
                    [128, 512], f32, name="ctxs", tag=f"ctxs{hp}")
            cs = ctx_stage[(qvb, hp)]
            t3 = ctxH[hh].rearrange("p (qs w) -> p qs w", w=VW)
            rb = cxp.tile([128, 4], f32, name="rb", tag=f"rb{hh}")
            nc.vector.reciprocal_approx_fast(
                out=rb[:], in_=t3[:, :, 64:65].rearrange("p a b -> p (a b)"))
            nc.vector.tensor_mul(
                cs.rearrange("p (qs hh f) -> p qs hh f", qs=4, hh=2)[
                    :, :, hh, :],
                t3[:, :, 0:64],
                rb.rearrange("p (a b) -> p a b", b=1).broadcast_to(
                    (128, 4, 64)))

        def norm(qvb, hp):
            norm_hh(qvb, hp, 0)
            norm_hh(qvb, hp, 1)

        def transpose(qvb, hp, qs, tail=False):
            cs = ctx_stage[(qvb, hp)]
            lhsT = cs[:, 128 * qs:128 * (qs + 1)]
            tp = t_slot(128) if tail else p_slot(128)
            nc.tensor.matmul(tp, lhsT, id_sb[:], is_transpose=True)
            # after the last exp ACT is idle; split evacs across ACT and DVE
            dst = ctxT_sb[hp][:, 512 * qvb + 128 * qs:512 * qvb + 128 * (qs + 1)]
            if tail and qs % 2 == 0:
                nc.scalar.copy(dst, tp)
            else:
                nc.vector.tensor_copy(dst, tp)
            if qs == 3:
                del ctx_stage[(qvb, hp)]

        # after the final exp the score banks are free: the tail out-proj /
        # transposes rotate over 4 whole-bank psum slots instead of 2
        tail_slots = [lambda w: projAB[0][:, 0:w], lambda w: projAB[1][:, 0:w],
                      lambda w: sc_ps[0][:, 0:w], lambda w: sc_ps[0][:, 512:512 + w],
                      lambda w: sc_ps[1][:, 0:w], lambda w: sc_ps[1][:, 512:512 + w]]

        def t_slot(w):
            i = cnt["p"]; cnt["p"] += 1
            return tail_slots[i % 6](w)

        def out_proj(qvb, qs, tail=False):
            o_sb = osb.tile([128, D], bf16, name="o_sb")
            qcols = (512 * qvb + 128 * qs, 512 * qvb + 128 * (qs + 1))
            for dc in range(2):
                o_ps = t_slot(512) if tail else p_slot(512)
                for cn in range(2):
                    nc.tensor.matmul(
                        o_ps[:], ctxT_sb[cn][:, qcols[0]:qcols[1]],
                        wo_sb[cn][:, 512 * dc:512 * (dc + 1)],
                        start=(cn == 0), stop=(cn == 1))
                if tail and dc % 2 == 0:
                    nc.scalar.copy(o_sb[:, 512 * dc:512 * (dc + 1)], o_ps[:])
                else:
                    nc.vector.tensor_copy(o_sb[:, 512 * dc:512 * (dc + 1)],
                                          o_ps[:])
                # half-store right after its evac, alternating queues so the
                # final stores drain two DGE pipelines in parallel; the last
                # store goes on sync (no gpsimd Q7 launch on the tail)
                eng = nc.gpsimd if dc == 0 else nc.sync
                eng.dma_start(
                    out_d[qcols[0]:qcols[1], 512 * dc:512 * (dc + 1)],
                    o_sb[:, 512 * dc:512 * (dc + 1)])

        # ================= emission schedule =================
        # One global stream of 128 exp units ((qvb, hp) sweeps, kchunk
        # minor). ctx matmuls trail by 13 units during sweep 0 (V still
        # streaming), then catch up 2-per-unit to a lag of 2 so the tail
        # stays short.
        UNITS = [(s // 2, s % 2, c) for s in range(8) for c in range(NKC)]
        NU = len(UNITS)
        pre = {u: [] for u in range(NU + 16)}

        def at(u, fn, *a):
            pre[u].append((fn, a))

        def wo_load():
            for cn in range(2):
                nc.sync.dma_start(wo_sb[cn][:], woT_d[128 * cn:128 * (cn + 1), :])

        # PE warmup: keep the tensor engine busy (and its p-state ramp hot)
        # through the DMA-bound prologue; calibrated to end near Ks0 arrival
        wsc = sb.tile([128, 512], bf16)
        nc.vector.memset(wsc[:], 1.0)

        def warmup(n):
            # rotate over all 6 tail slots so the WAW chain never paces the
            # warmup below the engine rate
            for _ in range(n):
                wp = t_slot(512)
                nc.tensor.matmul(wp, wsc[:, 0:128], wsc[:], start=True, stop=True)

        # prologue DMAs (sync queue order = arrival order): hp0 weight
        # halves + K s0 + Q s0 form the minimal first-exp prefix
        w_load(wk_sb, wkP_d, 0)
        w_load(wq_sb, wqP_d, 0)
        first_load(KT_d, kin, "k")
        first_load(QT_d, qin, "q")
        nc.sync.dma_start(
            bq_sb.rearrange("p (hp c) -> p hp c", hp=2),
            bq_d.rearrange("(hp p) c -> p hp c", p=128))
        chunk_load(KT_d, kin, "k", 1)
        nc.sync.dma_start(id_sb[:], id_d[:, :])
        w_load(wv_sb, wvP_d)
        w_load(wk_sb, wkP_d, 1)
        w_load(wq_sb, wqP_d, 1)
        qk_proj("k", wk_sb, kT_sb, 0, 0, False)
        qk_proj("k", wk_sb, kT_sb, 0, 1, False)
        qk_proj("q", wq_sb, qT_sb, 0, 0, True)
        qk_proj("q", wq_sb, qT_sb, 0, 1, True)

        at(2, chunk_load, KT_d, kin, "k", 2)
        at(2, qk_proj, "k", wk_sb, kT_sb, 0, 2, False)
        at(3, chunk_load, KT_d, kin, "k", 3)
        at(3, qk_proj, "k", wk_sb, kT_sb, 0, 3, False)
        at(4, chunk_load, VT_d, vin, "v", 0)
        at(5, chunk_load, VT_d, vin, "v", 1)
        at(5, qk_proj, "k", wk_sb, kT_sb, 0, 4, False)
        at(6, qk_proj, "k", wk_sb, kT_sb, 0, 5, False)
        at(6, chunk_load, VT_d, vin, "v", 2)
        at(7, qk_proj, "q", wq_sb, qT_sb, 1, 0, True)
        at(7, chunk_load, VT_d, vin, "v", 3)
        at(8, qk_proj, "k", wk_sb, kT_sb, 0, 6, False)
        at(9, qk_proj, "k", wk_sb, kT_sb, 0, 7, False)
        at(9, chunk_load, QT_d, qin, "q", 1)
        at(10, qk_proj, "q", wq_sb, qT_sb, 1, 1, True)
        at(10, wo_load)
        at(11, chunk_load, QT_d, qin, "q", 2)
        at(11, qk_proj, "k", wk_sb, kT_sb, 1, 0, False)
        at(12, qk_proj, "k", wk_sb, kT_sb, 1, 1, False)
        at(44, chunk_load, QT_d, qin, "q", 3)
        at(13, qk_proj, "k", wk_sb, kT_sb, 1, 2, False)
        at(14, qk_proj, "k", wk_sb, kT_sb, 1, 3, False)
        at(15, qk_proj, "k", wk_sb, kT_sb, 1, 4, False)
        at(18, qk_proj, "k", wk_sb, kT_sb, 1, 5, False)
        at(21, qk_proj, "k", wk_sb, kT_sb, 1, 6, False)
        at(24, qk_proj, "k", wk_sb, kT_sb, 1, 7, False)
        at(26, qk_proj, "q", wq_sb, qT_sb, 0, 2, True)
        at(28, qk_proj, "q", wq_sb, qT_sb, 0, 3, True)
        at(40, qk_proj, "q", wq_sb, qT_sb, 1, 2, True)
        at(42, qk_proj, "q", wq_sb, qT_sb, 1, 3, True)
        at(56, qk_proj, "q", wq_sb, qT_sb, 0, 4, True)
        at(58, qk_proj, "q", wq_sb, qT_sb, 0, 5, True)
        at(72, qk_proj, "q", wq_sb, qT_sb, 1, 4, True)
        at(74, qk_proj, "q", wq_sb, qT_sb, 1, 5, True)
        at(88, qk_proj, "q", wq_sb, qT_sb, 0, 6, True)
        at(90, qk_proj, "q", wq_sb, qT_sb, 0, 7, True)
        at(104, qk_proj, "q", wq_sb, qT_sb, 1, 6, True)
        at(106, qk_proj, "q", wq_sb, qT_sb, 1, 7, True)
        # out-proj spread through the back half to keep the PE backlog alive
        # (first use must follow the (qvb, hp1) sweep drain at ~32qvb+35)
        for qvb in range(3):
            for qs in range(4):
                at(58 + 26 * qvb + 6 * qs, out_proj, qvb, qs, False)

        j = 0  # ctx stream pointer into UNITS

        def ctx_lag(i):
            # sweep 0 trails 13..18 units (V still streaming; spreads the
            # vproj work), later sweeps 2; the 1.5-per-unit catch-up in the
            # emission loop decays the lag smoothly
            if i < NKC:
                return 13 + i
            return 1 if i >= NU - NKC else 2

        # within a unit: injections and trailing ctx work go BEFORE the
        # scores pair — the in-order PE can chew on them while waiting for
        # exp(u-2) to free the scores psum tag. Mid-stream drain work (hh1
        # norm + transposes) defers 2-items-per-unit so it never wedges the
        # scores stream.
        def final_drain():
            # last sweep: ctx c15, batched norm, transposes, then dc-major
            # out-proj over the 6-slot tail rotation
            qvb, hp, c = UNITS[NU - 1]
            ctx_mm(qvb, hp, c)
            norm(qvb, hp)
            for qs in range(4):
                transpose(qvb, hp, qs, tail=True)
            o_sbs = [osb.tile([128, D], bf16, name="o_sb") for _ in range(4)]
            for dc in range(2):
                for qs in range(4):
                    qc = (512 * qvb + 128 * qs, 512 * qvb + 128 * (qs + 1))
                    o_ps = t_slot(512)
                    for cn in range(2):
                        nc.tensor.matmul(
                            o_ps[:], ctxT_sb[cn][:, qc[0]:qc[1]],
                            wo_sb[cn][:, 512 * dc:512 * (dc + 1)],
                            start=(cn == 0), stop=(cn == 1))
                    dst = o_sbs[qs][:, 512 * dc:512 * (dc + 1)]
                    if (2 * dc + qs) % 2 == 0:
                        nc.scalar.copy(dst, o_ps[:])
                    else:
                        nc.vector.tensor_copy(dst, o_ps[:])
                    eng = nc.sync if (dc + qs) % 2 == 0 else nc.gpsimd
                    eng.dma_start(
                        out_d[qc[0]:qc[1], 512 * dc:512 * (dc + 1)], dst)

        deferred = []
        for u in range(NU + 24):
            for fn, a in pre[u]:
                fn(*a)
            for _ in range(2):
                if deferred:
                    deferred.pop(0)()
            emitted = 0
            cap = 2 if u % 2 else 1
            while j < NU and j <= u - ctx_lag(j) and emitted < cap:
                qv2, hp2, c2 = UNITS[j]
                if j == NU - 1:
                    final_drain()
                    j += 1
                    break
                if j < NKC:
                    v_proj(c2)
                ctx_mm(qv2, hp2, c2)
                emitted += 1
                j += 1
                if c2 == NKC - 1:
                    norm_hh(qv2, hp2, 0)
                    deferred.append(
                        lambda q=qv2, h=hp2: norm_hh(q, h, 1))
                    for qs in range(4):
                        deferred.append(
                            lambda q=qv2, h=hp2, s=qs: transpose(q, h, s))
                    break  # don't cross a drain inside one unit
            if u < NU:
                scores_exp(*UNITS[u])
            if u >= NU and j >= NU:
                break

    nc.compile()
    return nc


def kernel(Q, K, V, wq, bq, wk, bk, wv, bv, wo, bo):
    import ml_dtypes
    from concourse.bass_utils import run_bass_kernel_spmd

    if "nc" not in _CACHE:
        _CACHE["nc"] = _build()
    nc = _CACHE["nc"]

    bf = ml_dtypes.bfloat16
    Q = np.asarray(Q, np.float32)
    K = np.asarray(K, np.float32)
    V = np.asarray(V, np.float32)
    QT = [np.ascontiguousarray(Q[b].T).astype(bf) for b in range(B)]
    KT = [np.ascontiguousarray(K[b].T).astype(bf) for b in range(B)]
    VT = [np.ascontiguousarray(V[b].T).astype(bf) for b in range(B)]
    def perm_qk(w, g):
        # [D, R] -> [128p, (hp, d, 128r)] with element [p,hp,d,r] =
        # wT[d*128+p, hp*128+r]
        wT = np.asarray(w, np.float32)[g * R:(g + 1) * R].T
        return np.ascontiguousarray(
            wT.reshape(8, 128, 2, 128).transpose(1, 2, 0, 3).reshape(128, 2048)
        ).astype(bf)

    def perm_v(w, g):
        wT = np.asarray(w, np.float32)[g * R:(g + 1) * R].T
        return np.ascontiguousarray(
            wT.reshape(8, 128, 256).transpose(1, 0, 2).reshape(128, 2048)
        ).astype(bf)

    wqP = [perm_qk(wq, g) for g in range(4)]
    wkP = [perm_qk(wk, g) for g in range(4)]
    wvP = [perm_v(wv, g) for g in range(4)]
    woT = [np.ascontiguousarray(np.asarray(wo, np.float32)[:, g * R:(g + 1) * R].T
                                ).astype(bf) for g in range(4)]
    bqs = [np.ascontiguousarray(np.asarray(bq, np.float32)[g * R:(g + 1) * R, None])
           for g in range(4)]
    ident = np.eye(128, dtype=np.float32)

    in_maps = []
    for c in range(NCORES):
        b, g = c // 4, c % 4
        in_maps.append({
            "QT": QT[b], "KT": KT[b], "VT": VT[b],
            "wqP": wqP[g], "wkP": wkP[g], "wvP": wvP[g], "woT": woT[g],
            "bq": bqs[g], "ident": ident,
        })

    global _LAST_IN_MAPS
    _LAST_IN_MAPS = in_maps
    res = run_bass_kernel_spmd(nc, in_maps, core_ids=list(range(NCORES)))

    host_bias = (np.asarray(bv, np.float32) @ np.asarray(wo, np.float32).T
                 + np.asarray(bo, np.float32))
    out = np.zeros((B, S, D), np.float32)
    for c in range(NCORES):
        out[c // 4] += np.asarray(res.results[c]["OUT"], np.float32)
    out += host_bias[None, None, :]
    return out


# revision 83
# speedup vs baseline: 1.0437x; 1.0040x over previous
"""MultiHeadAttention TRN2 kernel: B=2, S=2048, D=1024, H=16, DK=64, 8 cores.

Sharding: core c handles batch b=c//4 and heads hg=(c%4)*4 .. +3 (data + head
parallel). Projections are column-split by head; out-proj row-split; the
all-reduce after out-proj is done on host (sum of 4 partials per batch).

All activations/weights stream HBM<->SBUF as bf16 (host converts), halving
DMA on the serial DMA-engine resource. Matmul inputs are bf16 (1 cycle/row at
any moving size) except qT/kT which stay f32r for exp-input precision.

Device dataflow (per core):
  qT/kT = (w-slice).T @ QT/KT      -> [feat 128 (2 heads), seq] f32r, 256-wide
  v     = VT.T @ wv-slice          -> natural [kpos, 4*64] chunks -> v_all bf16
                                      [kpos, head*16*65] with ones col (den)
  scoresT[kpos,q] = kT-chunk.T @ qT  (K=64, both heads packed in one
                                      [128,1024] 2-bank PSUM tile)
  expT = exp(scoresT/8)            -> bf16 SBUF (ACT, the throughput floor)
  ctx[q, 65] += expT-slice.T @ v-chunk   (F=65 bf16, PSUM accum over kpos;
                                          col 64 accumulates the denominator)
  ctx_sb = ctx * recip(den)        -> [q, 128(2 heads)] f32 per qsub
  ctxT = PE-transpose(ctx_sb)      -> ctxT_sb [feat, q] bf16
  out[q, Dout] = ctxT.T @ wo       -> 256-wide chunks -> bf16 partial out

The whole thing is software-pipelined at DMA-chunk granularity: the hp0
weight halves + K s0 + Q s0 form a minimal first-exp prefix on the sync
queue (first exp at ~12us), then the remaining K/V/Q chunks stream in while
one global stream of 128 scores->exp units runs; ctx matmuls trail the exp
stream by 13..18 units during sweep 0 (V still arriving) then catch up to a
lag of 2; projections / out-proj / norms / transposes are injected between
units so the in-order PE never waits on data that hasn't arrived. The exp
stream runs back-to-back (1038 ns/tile) through the ACT-bound middle.

PSUM (8 banks): scA/scB [128,1024] x2 banks each (scores/exp dbuf) | ctxA,
ctxB [128,455] (7 of 8 per-sweep [128,65] accumulators, parity-alternating) |
ctxC [128,512] (8th accumulator per parity) | projC [128,512] (2 rotating
256-wide slots for q/k/v-proj, out-proj and ctx transposes).

Bias handling (exact): bq added on device (per-partition add in qT layout);
bk dropped (softmax shift-invariance); bv and bo folded on host as
out += bv @ wo.T + bo (softmax weights sum to 1).
"""

from contextlib import ExitStack

import numpy as np

B, S, D, H, DK = 2, 2048, 1024, 16, 64
NCORES = 8
HPC = H // (NCORES // B)      # heads per core = 4
R = HPC * DK                  # local feats = 256
NKC = S // 128                # 128-wide k chunks = 16
VW = 65                       # v chunk width (64 + ones col)

_CACHE = {}
_LAST_IN_MAPS = None


def _build():
    import concourse.mybir as mybir
    import concourse.tile as tile
    from concourse import bacc

    f32 = mybir.dt.float32
    f32r = mybir.dt.float32r
    bf16 = mybir.dt.bfloat16
    Exp = mybir.ActivationFunctionType.Exp
    Add = mybir.AluOpType.add
    Mult = mybir.AluOpType.mult

    nc = bacc.Bacc(
        "TRN2", target_bir_lowering=False, debug=False,
        enable_asserts=True, num_devices=NCORES,
    )

    QT_d = nc.dram_tensor("QT", [D, S], bf16, kind="ExternalInput").ap()
    KT_d = nc.dram_tensor("KT", [D, S], bf16, kind="ExternalInput").ap()
    VT_d = nc.dram_tensor("VT", [D, S], bf16, kind="ExternalInput").ap()
    # weights host-permuted to exact SBUF layout: wq/wk as [128, hp, d, 128]
    # (per-hp halves load separately, 2KB-contiguous rows), wv as [128, d, 256]
    wqP_d = nc.dram_tensor("wqP", [128, 2 * 8 * 128], bf16,
                           kind="ExternalInput").ap()
    wkP_d = nc.dram_tensor("wkP", [128, 2 * 8 * 128], bf16,
                           kind="ExternalInput").ap()
    wvP_d = nc.dram_tensor("wvP", [128, 8 * 256], bf16,
                           kind="ExternalInput").ap()
    woT_d = nc.dram_tensor("woT", [R, D], bf16, kind="ExternalInput").ap()
    bq_d = nc.dram_tensor("bq", [R, 1], f32, kind="ExternalInput").ap()
    id_d = nc.dram_tensor("ident", [128, 128], f32, kind="ExternalInput").ap()
    out_d = nc.dram_tensor("OUT", [S, D], bf16, kind="ExternalOutput").ap()

    with tile.TileContext(nc) as tc, ExitStack() as ctx:
        sb = ctx.enter_context(tc.tile_pool(name="sb", bufs=1))
        qin = ctx.enter_context(tc.tile_pool(name="qin", bufs=6))
        kin = ctx.enter_context(tc.tile_pool(name="kin", bufs=8))
        vin = ctx.enter_context(tc.tile_pool(name="vin", bufs=6))
        expp = ctx.enter_context(tc.tile_pool(name="expp", bufs=31))
        cxp = ctx.enter_context(tc.tile_pool(name="cxp", bufs=2))
        osb = ctx.enter_context(tc.tile_pool(name="osb", bufs=4))
        psum = ctx.enter_context(tc.tile_pool(name="psum", bufs=1, space="PSUM"))

        # ---- persistent PSUM containers (8 banks exactly) ----
        # PSUM accumulation groups are zero-region (= bank) granular: a
        # start_tensor_calc matmul zeroes its whole bank, so each bank holds
        # exactly one live group. Scores halves are full banks; the 4 ctx
        # accumulators of one hh live in one bank as a single group; proj /
        # out-proj / transpose rotate through two whole-bank slots.
        sc_ps = [psum.tile([128, 1024], f32, name=f"sc{i}") for i in range(2)]
        ctxH = [psum.tile([128, 260], f32, name=f"ctxh{i}") for i in range(2)]
        projAB = [psum.tile([128, 512], f32, name=f"proj{i}") for i in range(2)]

        cnt = {"p": 0}

        def p_slot(w):
            # rotating whole-bank psum slot for q/k/v-proj, out-proj and
            # transposes; overlapping-view hazards serialize reuse
            i = cnt["p"]; cnt["p"] += 1
            return projAB[i % 2][:, 0:w]

        # ---- persistent SBUF ----
        wq_sb = sb.tile([128, 8 * R], bf16)   # (hp, d) block at 1024*hp+128*d
        wk_sb = sb.tile([128, 8 * R], bf16)
        wv_sb = sb.tile([128, 8 * R], bf16)   # D-chunk d at cols [R*d : +R]
        wo_sb = [sb.tile([128, D], bf16, name=f"wo_sb{cn}") for cn in range(2)]
        bq_sb = sb.tile([128, 2], f32)
        id_sb = sb.tile([128, 128], f32)

        qT_sb = [sb.tile([128, S], bf16, name=f"qT_sb{hp}") for hp in range(2)]
        kT_sb = [sb.tile([128, S], bf16, name=f"kT_sb{hp}") for hp in range(2)]
        v_all = sb.tile([128, HPC * NKC * VW], bf16)  # (h, c) at (h*NKC+c)*VW
        ctxT_sb = [sb.tile([128, S], bf16, name=f"ctxT_sb{cn}") for cn in range(2)]

        onecol = sb.tile([128, 1], f32)
        nc.vector.memset(onecol[:], 1.0)
        vv = v_all.rearrange("p (n c) -> p n c", c=VW)[:, :, 64:65].rearrange(
            "p n c -> p (n c)")
        nc.vector.tensor_copy(vv, onecol[:].broadcast_to((128, HPC * NKC)))

        def w_load(w_sb, w_d, hp=None):
            if hp is None:
                nc.sync.dma_start(w_sb[:], w_d[:, :])
            else:
                nc.sync.dma_start(w_sb[:, 1024 * hp:1024 * (hp + 1)],
                                  w_d[:, 1024 * hp:1024 * (hp + 1)])

        # staging tiles: one [128, 2048] bf16 tile covers 4 d-chunks x 512
        # seq; a (tensor, sblk) pair = 2 tiles (d 0-3, d 4-7)
        stage = {}

        def chunk_load(src, pool, tag, sblk):
            tiles = []
            for hf in range(2):
                t = pool.tile([128, 2048], bf16, name=tag, tag=tag)
                nc.sync.dma_start(
                    t.rearrange("p (d s) -> p d s", d=4),
                    src.rearrange("(d p) s -> p d s", p=128)[
                        :, 4 * hf:4 * hf + 4, 512 * sblk:512 * (sblk + 1)])
                tiles.append(t)
            stage[(tag, sblk)] = ("std", tiles)

        def first_load(src, pool, tag):
            # s0 of K/Q as two all-d x 256-col chunks so the first proj
            # tile only waits for half the data
            tiles = []
            for half in range(2):
                t = pool.tile([128, 2048], bf16, name=tag, tag=tag)
                nc.sync.dma_start(
                    t.rearrange("p (d s) -> p d s", d=8),
                    src.rearrange("(d p) s -> p d s", p=128)[
                        :, :, 256 * half:256 * (half + 1)])
                tiles.append(t)
            stage[(tag, 0)] = ("first", tiles)

        def staged(tag, sblk, d, cols):
            # d-chunk d of sblk, column slice `cols` within the 512-wide sblk
            ent = stage[(tag, sblk)]
            if ent[0] == "first":
                half = cols[0] // 256
                off = cols[0] - 256 * half
                return ent[1][half][
                    :, 256 * d + off:256 * d + off + cols[1] - cols[0]]
            t = ent[1][d // 4]
            base = 512 * (d % 4)
            return t[:, base + cols[0]:base + cols[1]]

        # ---- projection tiles ----
        def qk_proj(tag, w_sb, dst_sb, hp, j, bias):
            # (hp, j): 256 seq cols [256j : 256j+256] of head-pair hp
            sblk, half = j // 2, j % 2
            cols = (256 * half, 256 * half + 256)
            p_ps = p_slot(256)
            for d in range(8):
                nc.tensor.matmul(
                    p_ps[:],
                    w_sb[:, 1024 * hp + 128 * d:1024 * hp + 128 * (d + 1)],
                    staged(tag, sblk, d, cols), start=(d == 0), stop=(d == 7))
            dst = dst_sb[hp][:, 256 * j:256 * (j + 1)]
            if bias:
                nc.vector.tensor_scalar(
                    dst, p_ps[:], bq_sb[:, hp:hp + 1], None, op0=Add)
            else:
                nc.vector.tensor_copy(dst, p_ps[:])

        def v_proj(c):
            # kpos chunk c (128 rows): out [kpos, 256 feats] -> v_all slices
            sblk, sub = c // 4, c % 4
            cols = (128 * sub, 128 * sub + 128)
            v_ps = p_slot(256)
            for d in range(8):
                nc.tensor.matmul(
                    v_ps[:], staged("v", sblk, d, cols),
                    wv_sb[:, R * d:R * (d + 1)], start=(d == 0), stop=(d == 7))
            va = v_all.rearrange("p (h n c) -> p h n c", h=HPC, n=NKC)
            nc.vector.tensor_copy(
                va[:, :, c:c + 1, 0:64],
                v_ps[:].rearrange("p (h n c) -> p h n c", h=HPC, n=1))

        # ---- attention sweep pieces (scores/exp stream + trailing ctx) ----
        exp_ring = {}

        def scores_exp(qvb, hp, c):
            s_ps = sc_ps[c % 2]
            for hh in range(2):
                nc.tensor.matmul(
                    s_ps[:, 512 * hh:512 * (hh + 1)],
                    kT_sb[hp][64 * hh:64 * (hh + 1), 128 * c:128 * (c + 1)],
                    qT_sb[hp][64 * hh:64 * (hh + 1), 512 * qvb:512 * (qvb + 1)],
                    start=True, stop=True)
            expT = expp.tile([128, 1024], bf16, name="expT")
            nc.scalar.activation(expT[:], s_ps[:], Exp, scale=0.125)
            exp_ring[(qvb, hp, c)] = expT

        def ctx_mm(qvb, hp, c):
            expT = exp_ring.pop((qvb, hp, c))
            for hh in range(2):
                gh = HPC // 2 * hp + hh
                for qs in range(4):
                    nc.tensor.matmul(
                        ctxH[hh][0:128, 65 * qs:65 * qs + VW],
                        expT[:, 512 * hh + 128 * qs:512 * hh + 128 * (qs + 1)],
                        v_all[:, (gh * NKC + c) * VW:(gh * NKC + c + 1) * VW],
                        start=(c == 0 and qs == 0),
                        stop=(c == NKC - 1 and qs == 3))

        ctx_stage = {}

        def norm_hh(qvb, hp, hh):
            # drain one ctxH bank: a strided recip over the 4 denominator
            # columns + one strided multiply into the (qs, hh, 64) staging
            # tile cs
            if hh == 0:
                ctx_stage[(qvb, hp)] = cxp.tile(
                    [128, 512], f32, name="ctxs", tag=f"ctxs{hp}")
            cs = ctx_stage[(qvb, hp)]
            t3 = ctxH[hh].rearrange("p (qs w) -> p qs w", w=VW)
            rb = cxp.tile([128, 4], f32, name="rb", tag=f"rb{hh}")
            nc.vector.reciprocal_approx_fast(
                out=rb[:], in_=t3[:, :, 64:65].rearrange("p a b -> p (a b)"))
            nc.vector.tensor_mul(
                cs.rearrange("p (qs hh f) -> p qs hh f", qs=4, hh=2)[
                    :, :, hh, :],
                t3[:, :, 0:64],
                rb.rearrange("p (a b) -> p a b", b=1).broadcast_to(
                    (128, 4, 64)))

        def norm(qvb, hp):
            norm_hh(qvb, hp, 0)
            norm_hh(qvb, hp, 1)

        def transpose(qvb, hp, qs, tail=False):
            cs = ctx_stage[(qvb, hp)]
            lhsT = cs[:, 128 * qs:128 * (qs + 1)]
            tp = t_slot(128) if tail else p_slot(128)
            nc.tensor.matmul(tp, lhsT, id_sb[:], is_transpose=True)
            # after the last exp ACT is idle; split evacs across ACT and DVE
            dst = ctxT_sb[hp][:, 512 * qvb + 128 * qs:512 * qvb + 128 * (qs + 1)]
            if tail and qs % 2 == 0:
                nc.scalar.copy(dst, tp)
            else:
                nc.vector.tensor_copy(dst, tp)
            if qs == 3:
                del ctx_stage[(qvb, hp)]

        # after the final exp the score banks are free: the tail out-proj /
        # transposes rotate over 4 whole-bank psum slots instead of 2
        tail_slots = [lambda w: projAB[0][:, 0:w], lambda w: projAB[1][:, 0:w],
                      lambda w: sc_ps[0][:, 0:w], lambda w: sc_ps[0][:, 512:512 + w],
                      lambda w: sc_ps[1][:, 0:w], lambda w: sc_ps[1][:, 512:512 + w]]

        def t_slot(w):
            i = cnt["p"]; cnt["p"] += 1
            return tail_slots[i % 6](w)

        def out_proj(qvb, qs, tail=False):
            o_sb = osb.tile([128, D], bf16, name="o_sb")
            qcols = (512 * qvb + 128 * qs, 512 * qvb + 128 * (qs + 1))
            for dc in range(2):
                o_ps = t_slot(512) if tail else p_slot(512)
                for cn in range(2):
                    nc.tensor.matmul(
                        o_ps[:], ctxT_sb[cn][:, qcols[0]:qcols[1]],
                        wo_sb[cn][:, 512 * dc:512 * (dc + 1)],
                        start=(cn == 0), stop=(cn == 1))
                if tail and dc % 2 == 0:
                    nc.scalar.copy(o_sb[:, 512 * dc:512 * (dc + 1)], o_ps[:])
                else:
                    nc.vector.tensor_copy(o_sb[:, 512 * dc:512 * (dc + 1)],
                                          o_ps[:])
                # half-store right after its evac, alternating queues so the
                # final stores drain two DGE pipelines in parallel; the last
                # store goes on sync (no gpsimd Q7 launch on the tail)
                eng = nc.gpsimd if dc == 0 else nc.sync
                eng.dma_start(
                    out_d[qcols[0]:qcols[1], 512 * dc:512 * (dc + 1)],
                    o_sb[:, 512 * dc:512 * (dc + 1)])

        # ================= emission schedule =================
        # One global stream of 128 exp units ((qvb, hp) sweeps, kchunk
        # minor). ctx matmuls trail by 13 units during sweep 0 (V still
        # streaming), then catch up 2-per-unit to a lag of 2 so the tail
        # stays short.
        UNITS = [(s // 2, s % 2, c) for s in range(8) for c in range(NKC)]
        NU = len(UNITS)
        pre = {u: [] for u in range(NU + 16)}

        def at(u, fn, *a):
            pre[u].append((fn, a))

        def wo_load():
            for cn in range(2):
                nc.sync.dma_start(wo_sb[cn][:], woT_d[128 * cn:128 * (cn + 1), :])

        # PE warmup: keep the tensor engine busy (and its p-state ramp hot)
        # through the DMA-bound prologue; calibrated to end near Ks0 arrival
        wsc = sb.tile([128, 512], bf16)
        nc.vector.memset(wsc[:], 1.0)

        def warmup(n):
            # rotate over all 6 tail slots so the WAW chain never paces the
            # warmup below the engine rate
            for _ in range(n):
                wp = t_slot(512)
                nc.tensor.matmul(wp, wsc[:, 0:128], wsc[:], start=True, stop=True)

        # prologue DMAs (sync queue order = arrival order): hp0 weight
        # halves + K s0 + Q s0 form the minimal first-exp prefix
        w_load(wk_sb, wkP_d, 0)
        w_load(wq_sb, wqP_d, 0)
        first_load(KT_d, kin, "k")
        first_load(QT_d, qin, "q")
        nc.sync.dma_start(
            bq_sb.rearrange("p (hp c) -> p hp c", hp=2),
            bq_d.rearrange("(hp p) c -> p hp c", p=128))
        chunk_load(KT_d, kin, "k", 1)
        nc.sync.dma_start(id_sb[:], id_d[:, :])
        w_load(wv_sb, wvP_d)
        w_load(wk_sb, wkP_d, 1)
        w_load(wq_sb, wqP_d, 1)
        qk_proj("k", wk_sb, kT_sb, 0, 0, False)
        qk_proj("k", wk_sb, kT_sb, 0, 1, False)
        qk_proj("q", wq_sb, qT_sb, 0, 0, True)
        qk_proj("q", wq_sb, qT_sb, 0, 1, True)

        at(2, chunk_load, KT_d, kin, "k", 2)
        at(2, qk_proj, "k", wk_sb, kT_sb, 0, 2, False)
        at(3, chunk_load, KT_d, kin, "k", 3)
        at(3, qk_proj, "k", wk_sb, kT_sb, 0, 3, False)
        at(4, chunk_load, VT_d, vin, "v", 0)
        at(5, chunk_load, VT_d, vin, "v", 1)
        at(5, qk_proj, "k", wk_sb, kT_sb, 0, 4, False)
        at(6, qk_proj, "k", wk_sb, kT_sb, 0, 5, False)
        at(6, chunk_load, VT_d, vin, "v", 2)
        at(7, qk_proj, "q", wq_sb, qT_sb, 1, 0, True)
        at(7, chunk_load, VT_d, vin, "v", 3)
        at(8, qk_proj, "k", wk_sb, kT_sb, 0, 6, False)
        at(9, qk_proj, "k", wk_sb, kT_sb, 0, 7, False)
        at(9, chunk_load, QT_d, qin, "q", 1)
        at(10, qk_proj, "q", wq_sb, qT_sb, 1, 1, True)
        at(10, wo_load)
        at(11, chunk_load, QT_d, qin, "q", 2)
        at(11, qk_proj, "k", wk_sb, kT_sb, 1, 0, False)
        at(12, qk_proj, "k", wk_sb, kT_sb, 1, 1, False)
        at(44, chunk_load, QT_d, qin, "q", 3)
        at(13, qk_proj, "k", wk_sb, kT_sb, 1, 2, False)
        at(14, qk_proj, "k", wk_sb, kT_sb, 1, 3, False)
        at(15, qk_proj, "k", wk_sb, kT_sb, 1, 4, False)
        at(18, qk_proj, "k", wk_sb, kT_sb, 1, 5, False)
        at(21, qk_proj, "k", wk_sb, kT_sb, 1, 6, False)
        at(24, qk_proj, "k", wk_sb, kT_sb, 1, 7, False)
        at(26, qk_proj, "q", wq_sb, qT_sb, 0, 2, True)
        at(28, qk_proj, "q", wq_sb, qT_sb, 0, 3, True)
        at(40, qk_proj, "q", wq_sb, qT_sb, 1, 2, True)
        at(42, qk_proj, "q", wq_sb, qT_sb, 1, 3, True)
        at(56, qk_proj, "q", wq_sb, qT_sb, 0, 4, True)
        at(58, qk_proj, "q", wq_sb, qT_sb, 0, 5, True)
        at(72, qk_proj, "q", wq_sb, qT_sb, 1, 4, True)
        at(74, qk_proj, "q", wq_sb, qT_sb, 1, 5, True)
        at(88, qk_proj, "q", wq_sb, qT_sb, 0, 6, True)
        at(90, qk_proj, "q", wq_sb, qT_sb, 0, 7, True)
        at(104, qk_proj, "q", wq_sb, qT_sb, 1, 6, True)
        at(106, qk_proj, "q", wq_sb, qT_sb, 1, 7, True)
        # out-proj spread through the back half to keep the PE backlog alive
        # (first use must follow the (qvb, hp1) sweep drain at ~32qvb+35)
        for qvb in range(3):
            for qs in range(4):
                at(58 + 26 * qvb + 6 * qs, out_proj, qvb, qs, False)

        j = 0  # ctx stream pointer into UNITS

        def ctx_lag(i):
            # sweep 0 trails 13..18 units (V still streaming; spreads the
            # vproj work), later sweeps 2; the 1.5-per-unit catch-up in the
            # emission loop decays the lag smoothly
            if i < NKC:
                return 14 + i
            return 1 if i >= NU - NKC else 2

        # within a unit: injections and trailing ctx work go BEFORE the
        # scores pair — the in-order PE can chew on them while waiting for
        # exp(u-2) to free the scores psum tag. Mid-stream drain work (hh1
        # norm + transposes) defers 2-items-per-unit so it never wedges the
        # scores stream.
        def final_drain():
            # last sweep: ctx c15, batched norm, transposes, then dc-major
            # out-proj over the 6-slot tail rotation
            qvb, hp, c = UNITS[NU - 1]
            ctx_mm(qvb, hp, c)
            norm(qvb, hp)
            for qs in range(4):
                transpose(qvb, hp, qs, tail=True)
            o_sbs = [osb.tile([128, D], bf16, name="o_sb") for _ in range(4)]
            for dc in range(2):
                for qs in range(4):
                    qc = (512 * qvb + 128 * qs, 512 * qvb + 128 * (qs + 1))
                    o_ps = t_slot(512)
                    for cn in range(2):
                        nc.tensor.matmul(
                            o_ps[:], ctxT_sb[cn][:, qc[0]:qc[1]],
                            wo_sb[cn][:, 512 * dc:512 * (dc + 1)],
                            start=(cn == 0), stop=(cn == 1))
                    dst = o_sbs[qs][:, 512 * dc:512 * (dc + 1)]
                    if (2 * dc + qs) % 2 == 0:
                        nc.scalar.copy(dst, o_ps[:])
                    else:
                        nc.vector.tensor_copy(dst, o_ps[:])
                    eng = nc.sync if (dc + qs) % 2 == 0 else nc.gpsimd
                    eng.dma_start(
                        out_d[qc[0]:qc[1], 512 * dc:512 * (dc + 1)], dst)

        deferred = []
        for u in range(NU + 24):
            for fn, a in pre[u]:
                fn(*a)
            for _ in range(2):
                if deferred:
                    deferred.pop(0)()
            emitted = 0
            cap = 2 if u % 2 else 1
            while j < NU and j <= u - ctx_lag(j) and emitted < cap:
                qv2, hp2, c2 = UNITS[j]
                if j == NU - 1:
                    final_drain()
                    j += 1
                    break
                if j < NKC:
                    v_proj(c2)
                ctx_mm(qv2, hp2, c2)
                emitted += 1
                j += 1
                if c2 == NKC - 1:
                    norm_hh(qv2, hp2, 0)
                    deferred.append(
                        lambda q=qv2, h=hp2: norm_hh(q, h, 1))
                    for qs in range(4):
                        deferred.append(
                            lambda q=qv2, h=hp2, s=qs: transpose(q, h, s))
                    break  # don't cross a drain inside one unit
            if u < NU:
                scores_exp(*UNITS[u])
            if u >= NU and j >= NU:
                break

    nc.compile()
    return nc


def kernel(Q, K, V, wq, bq, wk, bk, wv, bv, wo, bo):
    import ml_dtypes
    from concourse.bass_utils import run_bass_kernel_spmd

    if "nc" not in _CACHE:
        _CACHE["nc"] = _build()
    nc = _CACHE["nc"]

    bf = ml_dtypes.bfloat16
    Q = np.asarray(Q, np.float32)
    K = np.asarray(K, np.float32)
    V = np.asarray(V, np.float32)
    QT = [np.ascontiguousarray(Q[b].T).astype(bf) for b in range(B)]
    KT = [np.ascontiguousarray(K[b].T).astype(bf) for b in range(B)]
    VT = [np.ascontiguousarray(V[b].T).astype(bf) for b in range(B)]
    def perm_qk(w, g):
        # [D, R] -> [128p, (hp, d, 128r)] with element [p,hp,d,r] =
        # wT[d*128+p, hp*128+r]
        wT = np.asarray(w, np.float32)[g * R:(g + 1) * R].T
        return np.ascontiguousarray(
            wT.reshape(8, 128, 2, 128).transpose(1, 2, 0, 3).reshape(128, 2048)
        ).astype(bf)

    def perm_v(w, g):
        wT = np.asarray(w, np.float32)[g * R:(g + 1) * R].T
        return np.ascontiguousarray(
            wT.reshape(8, 128, 256).transpose(1, 0, 2).reshape(128, 2048)
        ).astype(bf)

    wqP = [perm_qk(wq, g) for g in range(4)]
    wkP = [perm_qk(wk, g) for g in range(4)]
    wvP = [perm_v(wv, g) for g in range(4)]
    woT = [np.ascontiguousarray(np.asarray(wo, np.float32)[:, g * R:(g + 1) * R].T
                                ).astype(bf) for g in range(4)]
    bqs = [np.ascontiguousarray(np.asarray(bq, np.float32)[g * R:(g + 1) * R, None])
           for g in range(4)]
    ident = np.eye(128, dtype=np.float32)

    in_maps = []
    for c in range(NCORES):
        b, g = c // 4, c % 4
        in_maps.append({
            "QT": QT[b], "KT": KT[b], "VT": VT[b],
            "wqP": wqP[g], "wkP": wkP[g], "wvP": wvP[g], "woT": woT[g],
            "bq": bqs[g], "ident": ident,
        })

    global _LAST_IN_MAPS
    _LAST_IN_MAPS = in_maps
    res = run_bass_kernel_spmd(nc, in_maps, core_ids=list(range(NCORES)))

    host_bias = (np.asarray(bv, np.float32) @ np.asarray(wo, np.float32).T
                 + np.asarray(bo, np.float32))
    out = np.zeros((B, S, D), np.float32)
    for c in range(NCORES):
        out[c // 4] += np.asarray(res.results[c]["OUT"], np.float32)
    out += host_bias[None, None, :]
    return out
